# revision 15
# baseline (speedup 1.0000x reference)
"""GAT + GCN + classifier over a COO graph, distributed over 8 TRN2 NeuronCores.

v2 strategy (descriptor- and instruction-count driven):
  - Nodes dealt to 8 cores by (d_lo1, d_hi1) lexsort round-robin (balances
    both the per-core edge count and the per-tile degree profiles).
  - Phase A: every core builds the full gather table T[n] = [h(n) | a_s(n)]
    (bf16, 768B rows) from x^T tiles (host-pretransposed, so no on-device
    transposes); PSUM->SBUF casts alternate DVE/ACT, paired 2 tiles/instr.
  - GAT phase: per dst-tile slot-major dma_gather (lo/hi int16 halves),
    softmax fused into ~7 wide instructions (ACT Lrelu + Exp, DVE broadcast
    adds + reduces), weighted sum via broadcast multiply + pairwise tree.
    ELU + bias deferred to one bulk pass per 8-tile group.
  - GCN phase: u = dinv*(xg @ Wc) AllGathered, then TRANSPOSED dma_gather
    (lane-major columns) -> per-lane tensor_reduce gives xc^T directly ->
    ACT relu(+bc) -> classifier matmul without any transposes. dst nodes
    re-sorted per core by GCN degree profile (perm2) to cut slot padding.
  - log_softmax bulk at the end; host de-permutes rows (perm2 order).
"""
import sys

sys.path.insert(0, "/opt/trn_rl_repo")

import numpy as np
import ml_dtypes

import concourse.bass as bass
import concourse.bacc as bacc
import concourse.mybir as mybir
import concourse.tile as tile
from concourse.bass_utils import run_bass_kernel_spmd

# problem constants (hardcoded per contract)
N = 50000
E = 800000
F_IN = 128
H = 4
C = 64
HC = H * C          # 256
HID = 128
NCLASS = 10
NEG = 0.2

NCORES = 8
P = 128
NPC = N // NCORES   # 6250 nodes per core
TPC = 49            # tiles per core (49*128 = 6272 >= 6250)
S = TPC * P         # 6272 padded slots per core
SPLIT = 32768       # int16 gather index range per table half
XT = 391            # x tiles for table build (391*128 = 50048)
NPAD = XT * P       # 50048
NT_ROWS = 1 + NPAD + 1   # gather table rows: [dummy | nodes (+pad) | hi dummy]
HI_DUMMY = NT_ROWS - 1   # 50049
TABW = 384          # bf16 table row: 0:256 h, 256:260 a_s, 260:384 junk pad
ASD_NEG = -10000.0  # a_s marker for dummy rows (drives softmax weight to ~0)
NU_ROWS = NCORES * S     # 50176 u-table rows
U_LO_DUMMY = NPC         # row 6250 (core0 pad slot -> always zero)
U_HI_DUMMY = 7 * S + NPC # row 50154 (core7 pad slot)
GRP = 4             # GAT tiles per ELU/phase-C group

f32 = mybir.dt.float32
bf16 = mybir.dt.bfloat16
i16 = mybir.dt.int16


def _build_structures(edge_index):
    src = np.asarray(edge_index[0], dtype=np.int64)
    dst = np.asarray(edge_index[1], dtype=np.int64)
    src = np.concatenate([src, np.arange(N, dtype=np.int64)])
    dst = np.concatenate([dst, np.arange(N, dtype=np.int64)])
    deg = np.bincount(dst, minlength=N).astype(np.int64)
    dinv = (1.0 / np.sqrt(deg)).astype(np.float32)

    indptr = np.zeros(N + 1, np.int64)
    np.cumsum(deg, out=indptr[1:])

    # --- GAT split: table row of node n follows the interleaved phase-A
    #     write pattern (chunk i4 of 4 x-tiles: row = 1 + i4*512 + p*n_t + k)
    row_of = np.empty(NPAD, np.int64)
    for i4 in range((XT + 3) // 4):
        n_t = min(4, XT - i4 * 4)
        for k in range(n_t):
            cols = (i4 * 4 + k) * P + np.arange(P)
            row_of[cols] = 1 + i4 * 4 * P + np.arange(P) * n_t + k
    rr = row_of[src]
    hi1 = rr >= SPLIT
    d_hi1 = np.bincount(dst[hi1], minlength=N).astype(np.int64)
    d_lo1 = deg - d_hi1
    order1 = np.lexsort((hi1, dst))
    adj1 = rr[order1]  # table rows, grouped by dst, lo sources first

    # --- node -> core deal by (d_lo1, d_hi1) lexsort (matches tile profiles
    #     across cores), then per-core sort by the same keys ---
    key_order = np.lexsort((d_hi1, d_lo1))
    perm = np.empty((NCORES, NPC), np.int64)
    for c in range(NCORES):
        nodes = key_order[c::NCORES]
        k = np.lexsort((d_hi1[nodes], d_lo1[nodes]))
        perm[c] = nodes[k]
    pos = np.empty(N, np.int64)
    for c in range(NCORES):
        pos[perm[c]] = c * S + np.arange(NPC)

    # --- GCN split: u-table row of node n is pos[n] ---
    ps = pos[src]
    hi2 = ps >= SPLIT
    d_hi2 = np.bincount(dst[hi2], minlength=N).astype(np.int64)
    d_lo2 = deg - d_hi2
    order2 = np.lexsort((hi2, dst))
    adj2 = ps[order2]  # u-table positions, grouped by dst, lo first

    # --- per-core GCN re-sort (perm2): same node set, ordered by GCN keys ---
    perm2 = np.empty((NCORES, NPC), np.int64)
    for c in range(NCORES):
        nodes = perm[c]
        k = np.lexsort((d_hi2[nodes], d_lo2[nodes]))
        perm2[c] = nodes[k]

    # --- common (max across cores) per-tile slot profiles ---
    def tile_prof(dvals, pm):
        m = np.zeros((NCORES, S), np.int64)
        for c in range(NCORES):
            m[c, :NPC] = dvals[pm[c]]
        return m.reshape(NCORES, TPC, P).max(axis=(0, 2))

    Dlo = tile_prof(d_lo1, perm)
    Dhi = tile_prof(d_hi1, perm)
    D2lo = tile_prof(d_lo2, perm2)
    D2hi = tile_prof(d_hi2, perm2)

    def block(nodes, Dt, dcount, base, adj, shift, dummy, lane_major):
        """Padded [Dt*128] int index block for one tile."""
        if Dt == 0:
            return np.zeros(0, np.int64)
        nv = np.maximum(nodes, 0)
        cnt = np.where(nodes >= 0, dcount[nv], 0)
        sl = np.arange(Dt)
        ei = base[:, None] + sl[None, :]
        valid = sl[None, :] < cnt[:, None]
        vals = np.where(valid, adj[np.where(valid, ei, 0)] + shift, dummy)
        if lane_major:
            return vals.reshape(-1)       # position = lane*Dt + slot
        return vals.T.reshape(-1)         # position = slot*128 + lane

    def wrap16(flat):
        # position i -> [i % 16, i // 16], replicated to 128 partitions
        arr = flat.reshape(-1, 16).T
        return np.tile(arr, (8, 1))

    gat_idx = []
    gcn_idx = []
    for c in range(NCORES):
        nodes_pad = np.full(S, -1, np.int64)
        nodes_pad[:NPC] = perm[c]
        nodes_pad2 = np.full(S, -1, np.int64)
        nodes_pad2[:NPC] = perm2[c]
        cols1 = []
        cols2 = []
        for t in range(TPC):
            nodes = nodes_pad[t * P:(t + 1) * P]
            nv = np.maximum(nodes, 0)
            b_lo1 = indptr[nv]
            b_hi1 = indptr[nv] + d_lo1[nv]
            lo = block(nodes, Dlo[t], d_lo1, b_lo1, adj1, 0, 0, False)
            hi = block(nodes, Dhi[t], d_hi1, b_hi1, adj1, -SPLIT,
                       HI_DUMMY - SPLIT, False)
            assert lo.size == 0 or (0 <= lo.min() and lo.max() < SPLIT)
            assert hi.size == 0 or (0 <= hi.min() and hi.max() <= HI_DUMMY - SPLIT)
            cols1.append(wrap16(lo))
            cols1.append(wrap16(hi))

            nodes2 = nodes_pad2[t * P:(t + 1) * P]
            nv2 = np.maximum(nodes2, 0)
            b_lo2 = indptr[nv2]
            b_hi2 = indptr[nv2] + d_lo2[nv2]
            lo2 = block(nodes2, D2lo[t], d_lo2, b_lo2, adj2, 0, U_LO_DUMMY, False)
            hi2b = block(nodes2, D2hi[t], d_hi2, b_hi2, adj2, -SPLIT,
                         U_HI_DUMMY - SPLIT, False)
            assert lo2.size == 0 or (0 <= lo2.min() and lo2.max() < SPLIT)
            assert hi2b.size == 0 or (0 <= hi2b.min() and hi2b.max() < SPLIT)
            cols2.append(wrap16(lo2))
            cols2.append(wrap16(hi2b))
        gat_idx.append(np.concatenate(cols1, axis=1).astype(np.int16))
        gcn_idx.append(np.concatenate(cols2, axis=1).astype(np.int16))

    return dict(
        dinv=dinv, perm=perm, perm2=perm2,
        Dlo=Dlo.tolist(), Dhi=Dhi.tolist(),
        D2lo=D2lo.tolist(), D2hi=D2hi.tolist(),
        gat_idx=gat_idx, gcn_idx=gcn_idx,
    )


def _build_kernel(Dlo, Dhi, D2lo, D2hi, gat_cols, gcn_cols):
    nc = bacc.Bacc(None, num_devices=NCORES, num_swdge_queues=4)

    x_padT = nc.declare_dram_parameter("x_padT", [F_IN, NPAD], bf16, isOutput=False)
    x_permT = nc.declare_dram_parameter("x_permT", [F_IN, S], bf16, isOutput=False)
    dinv_pt = nc.declare_dram_parameter("dinv_pt", [P, TPC], f32, isOutput=False)
    dinv2_pt = nc.declare_dram_parameter("dinv2_pt", [P, TPC], f32, isOutput=False)
    gat_idx = nc.declare_dram_parameter("gat_idx", [P, gat_cols], i16, isOutput=False)
    gcn_idx = nc.declare_dram_parameter("gcn_idx", [P, gcn_cols], i16, isOutput=False)
    wg_aug = nc.declare_dram_parameter("wg_aug", [F_IN, TABW], bf16, isOutput=False)
    wg_ad = nc.declare_dram_parameter("wg_ad", [F_IN, H], bf16, isOutput=False)
    wc = nc.declare_dram_parameter("wc", [HC, HID], bf16, isOutput=False)
    wl = nc.declare_dram_parameter("wl", [HID, NCLASS], bf16, isOutput=False)
    bg_b = nc.declare_dram_parameter("bg_b", [P, HC], f32, isOutput=False)
    bc_b = nc.declare_dram_parameter("bc_b", [P, HID], f32, isOutput=False)
    bl_b = nc.declare_dram_parameter("bl_b", [P, NCLASS], f32, isOutput=False)
    ident_bf_in = nc.declare_dram_parameter("ident_bf", [P, P], bf16, isOutput=False)
    ident_f_in = nc.declare_dram_parameter("ident_f", [P, P], f32, isOutput=False)
    out = nc.declare_dram_parameter("out", [S, NCLASS], f32, isOutput=True)

    h_table = nc.dram_tensor("h_table", [NT_ROWS, TABW], bf16)
    ag_in = nc.dram_tensor("ag_in", [S, HID], bf16)
    ag_out = nc.dram_tensor("ag_out", [NU_ROWS, HID], bf16, addr_space="Shared")

    AW = TABW
    Act = mybir.ActivationFunctionType

    with tile.TileContext(nc) as tc:
        with (
            tc.tile_pool(name="const", bufs=1) as cpool,
            tc.tile_pool(name="sbuf", bufs=3) as sb,
            tc.tile_pool(name="gat", bufs=2) as gp,
            tc.tile_pool(name="scratch", bufs=1) as sp,
            tc.tile_pool(name="softmax", bufs=3) as spE,
            tc.tile_pool(name="gut", bufs=3) as gu,
            tc.tile_pool(name="psA", bufs=2, space="PSUM") as ppA,
            tc.tile_pool(name="psB", bufs=2, space="PSUM") as ppB,
        ):
            # ---- resident constants ----
            ident_bf = cpool.tile([P, P], bf16)
            nc.sync.dma_start(out=ident_bf[:], in_=ident_bf_in[:])
            wga_t = cpool.tile([F_IN, AW], bf16)
            nc.sync.dma_start(out=wga_t[:], in_=wg_aug[:])
            wgad_t = cpool.tile([F_IN, H], bf16)
            nc.sync.dma_start(out=wgad_t[:], in_=wg_ad[:])
            wc_t = cpool.tile([P, 2, HID], bf16)
            nc.sync.dma_start(out=wc_t[:], in_=wc.rearrange("(k p) n -> p k n", p=P))
            wlb_t = cpool.tile([HID, NCLASS], bf16)
            nc.sync.dma_start(out=wlb_t[:], in_=wl[:])
            bg_t = cpool.tile([P, HC], f32)
            nc.sync.dma_start(out=bg_t[:], in_=bg_b[:])
            bc_t = cpool.tile([P, HID], f32)
            nc.sync.dma_start(out=bc_t[:], in_=bc_b[:])
            ident_f = cpool.tile([P, P], f32)
            nc.sync.dma_start(out=ident_f[:], in_=ident_f_in[:])
            bl_t = cpool.tile([P, NCLASS], f32)
            nc.sync.dma_start(out=bl_t[:], in_=bl_b[:])
            dinv_t = cpool.tile([P, TPC], f32)
            nc.sync.dma_start(out=dinv_t[:], in_=dinv_pt[:])
            dinv2_t = cpool.tile([P, TPC], f32)
            nc.sync.dma_start(out=dinv2_t[:], in_=dinv2_pt[:])
            gcn_ix = cpool.tile([P, gcn_cols], i16)
            nc.scalar.dma_start(out=gcn_ix[:], in_=gcn_idx[:])
            ad_all = cpool.tile([P, TPC, H], f32)
            xg_all = cpool.tile([P, TPC, HC], bf16)
            lg_all = cpool.tile([P, TPC, NCLASS], f32)

            # ---- dummy table rows ----
            dum = cpool.tile([1, TABW], bf16)
            nc.vector.memset(dum[:], 0.0)
            nc.vector.memset(dum[:, HC:HC + H], ASD_NEG)
            nc.sync.dma_start(out=h_table[0:1, :], in_=dum[:])
            nc.sync.dma_start(out=h_table[HI_DUMMY:HI_DUMMY + 1, :], in_=dum[:])

            # ---- phase A: build gather table rows 1..NPAD ----
            # 4 tiles per chunk: one 1KB-contiguous x read and one 3KB-
            # contiguous interleaved table write (row of node = host pi map).
            for i4 in range((XT + 3) // 4):
                n_t = min(4, XT - i4 * 4)
                xq = sb.tile([P, 4 * P], bf16, tag="xq")
                nc.sync.dma_start(
                    out=xq[:, 0:n_t * P],
                    in_=x_padT[:, i4 * 4 * P:(i4 * 4 + n_t) * P],
                )
                hbf = sb.tile([P, 4, AW], bf16, tag="hbf")
                for k2 in range((n_t + 1) // 2):
                    n_p = min(2, n_t - k2 * 2)
                    hps = ppA.tile([P, 2, 512], f32, tag="mm_psA")
                    for k in range(n_p):
                        nc.tensor.matmul(
                            hps[:, k, 0:AW],
                            lhsT=xq[:, (k2 * 2 + k) * P:(k2 * 2 + k + 1) * P],
                            rhs=wga_t[:], start=True, stop=True,
                        )
                    src_ap = hps[:, 0:n_p, 0:AW]
                    dst_ap = hbf[:, k2 * 2:k2 * 2 + n_p, :]
                    if k2 % 2 == 0:
                        nc.vector.tensor_copy(out=dst_ap, in_=src_ap)
                    else:
                        nc.scalar.activation(out=dst_ap, in_=src_ap, func=Act.Copy)
                nc.sync.dma_start(
                    out=h_table[
                        1 + i4 * 4 * P:1 + (i4 * 4 + n_t) * P, :
                    ].rearrange("(p k) w -> p k w", k=n_t),
                    in_=hbf[:, 0:n_t, :],
                )

            # ---- phase A2: per-tile a_d for this core's own nodes ----
            for t in range(TPC):
                xT = sb.tile([P, P], bf16, tag="xT")
                nc.sync.dma_start(out=xT[:], in_=x_permT[:, t * P:(t + 1) * P])
                adps = ppB.tile([P, H], f32, tag="mm_psB")
                nc.tensor.matmul(adps[:], lhsT=xT[:], rhs=wgad_t[:],
                                 start=True, stop=True)
                nc.vector.tensor_copy(out=ad_all[:, t, :], in_=adps[:])

            # ---- phase B: GAT per tile; ELU + phase C per group of 8 ----
            qrr = [0]  # swdge queue round-robin
            goff = 0
            for g0 in range(0, TPC, GRP):
                g1 = min(g0 + GRP, TPC)
                for t in range(g0, g1):
                    dlo, dhi = Dlo[t], Dhi[t]
                    D = dlo + dhi
                    w = 8 * D
                    idx_t = gp.tile([P, w], i16, tag="gidx")
                    nc.scalar.dma_start(out=idx_t[:], in_=gat_idx[:, goff:goff + w])
                    goff += w
                    G = gp.tile([P, D, TABW], bf16, tag="G")
                    if dlo > 0:
                        nc.gpsimd.dma_gather(
                            out_ap=G[:, 0:dlo, :],
                            in_ap=h_table[:, :],
                            idxs_ap=idx_t[:, 0:8 * dlo],
                            num_idxs=P * dlo,
                            num_idxs_reg=P * dlo,
                            elem_size=TABW,
                            single_packet=False,
                            queue_num=qrr[0] % 4,
                        )
                        qrr[0] += 1
                    if dhi > 0:
                        nc.gpsimd.dma_gather(
                            out_ap=G[:, dlo:D, :],
                            in_ap=h_table[SPLIT:, :],
                            idxs_ap=idx_t[:, 8 * dlo:w],
                            num_idxs=P * dhi,
                            num_idxs_reg=P * dhi,
                            elem_size=TABW,
                            single_packet=False,
                            queue_num=qrr[0] % 4,
                        )
                        qrr[0] += 1
                    # e = leaky_relu(a_s[src] + a_d[dst])   [P, H, D]
                    e = spE.tile([P, H, D], f32, tag="e")
                    nc.vector.tensor_tensor(
                        out=e[:],
                        in0=G[:, :, HC:HC + H].rearrange("p d h -> p h d"),
                        in1=ad_all[:, t, :][:, :, None].to_broadcast([P, H, D]),
                        op=mybir.AluOpType.add,
                    )
                    e2 = spE.tile([P, H, D], f32, tag="e2")
                    nc.vector.tensor_scalar(
                        out=e2[:], in0=e[:], scalar1=NEG, scalar2=None,
                        op0=mybir.AluOpType.mult,
                    )
                    nc.vector.tensor_tensor(
                        out=e[:], in0=e[:], in1=e2[:], op=mybir.AluOpType.max
                    )
                    negm = spE.tile([P, H], f32, tag="negm")
                    nc.vector.tensor_reduce(
                        out=negm[:], in_=e[:], axis=mybir.AxisListType.X,
                        op=mybir.AluOpType.max, negate=True,
                    )
                    ex = spE.tile([P, H, D], bf16, tag="ex")
                    nc.vector.tensor_tensor(
                        out=ex[:], in0=e[:],
                        in1=negm[:, :, None].to_broadcast([P, H, D]),
                        op=mybir.AluOpType.add,
                    )
                    nc.scalar.activation(out=ex[:], in_=ex[:], func=Act.Exp)
                    den = spE.tile([P, H], f32, tag="den")
                    nc.vector.tensor_reduce(
                        out=den[:], in_=ex[:], axis=mybir.AxisListType.X,
                        op=mybir.AluOpType.add,
                    )
                    rden = spE.tile([P, H], f32, tag="rden")
                    nc.vector.reciprocal(rden[:], den[:])
                    # prod[p, d, h, c] = h_gathered * ex  (bf16)
                    prod = sp.tile([P, D, HC], bf16, tag="prod")
                    g_h = G[:, :, 0:HC].rearrange("p d (h c) -> p d h c", h=H)
                    ex_b = ex.rearrange("p h d -> p d h")[:, :, :, None].to_broadcast(
                        [P, D, H, C]
                    )
                    nc.vector.tensor_tensor(
                        out=prod.rearrange("p d (h c) -> p d h c", h=H),
                        in0=g_h, in1=ex_b, op=mybir.AluOpType.mult,
                    )
                    # tree-reduce over D slots, in place on prod (bf16
                    # until width <= 6, then one f32 level)
                    cur = D
                    while cur > 6:
                        h2 = cur // 2
                        nc.vector.tensor_tensor(
                            out=prod[:, 0:h2, :], in0=prod[:, 0:h2, :],
                            in1=prod[:, h2:2 * h2, :], op=mybir.AluOpType.add,
                        )
                        if cur % 2:
                            nc.vector.tensor_tensor(
                                out=prod[:, 0, :], in0=prod[:, 0, :],
                                in1=prod[:, 2 * h2, :], op=mybir.AluOpType.add,
                            )
                        cur = h2
                    acc = sp.tile([P, HC], f32, tag="accF")
                    if cur == 1:
                        nc.vector.tensor_copy(out=acc[:], in_=prod[:, 0, :])
                    else:
                        nc.vector.tensor_tensor(
                            out=acc[:], in0=prod[:, 0, :], in1=prod[:, 1, :],
                            op=mybir.AluOpType.add,
                        )
                        for j in range(2, cur):
                            nc.vector.tensor_tensor(
                                out=acc[:], in0=acc[:], in1=prod[:, j, :],
                                op=mybir.AluOpType.add,
                            )
                    # xg_raw = acc / den  -> bulk buffer (bf16)
                    nc.vector.tensor_tensor(
                        out=xg_all[:, t, :].rearrange("p (h c) -> p h c", h=H),
                        in0=acc.rearrange("p (h c) -> p h c", h=H),
                        in1=rden[:, :, None].to_broadcast([P, H, C]),
                        op=mybir.AluOpType.mult,
                    )

                # ---- group ELU: xg = elu(xg_raw + bg) in-place (bf16) ----
                ng = g1 - g0
                xs = xg_all[:, g0:g1, :]
                nc.vector.tensor_tensor(
                    out=xs, in0=xs,
                    in1=bg_t[:, None, :].to_broadcast([P, ng, HC]),
                    op=mybir.AluOpType.add,
                )
                tneg = sp.tile([P, GRP * HC], f32, tag="tneg")
                tn = tneg[:, 0:ng * HC].rearrange("p (g c) -> p g c", g=ng)
                nc.vector.tensor_scalar(
                    out=tn, in0=xs, scalar1=0.0, scalar2=None,
                    op0=mybir.AluOpType.min,
                )
                nc.scalar.activation(out=tn, in_=tn, func=Act.Exp)
                nc.vector.tensor_scalar(
                    out=xs, in0=xs, scalar1=0.0, scalar2=None,
                    op0=mybir.AluOpType.max,
                )
                nc.vector.tensor_tensor(
                    out=xs, in0=xs, in1=tn, op=mybir.AluOpType.add,
                )
                nc.vector.tensor_scalar(
                    out=xs, in0=xs, scalar1=1.0, scalar2=None,
                    op0=mybir.AluOpType.subtract,
                )

                # ---- phase C for the group: u = dinv * (xg @ Wc) ----
                for t in range(g0, g1):
                    xwps = ppB.tile([P, HID], f32, tag="mm_psB")
                    for k in range(2):
                        xgT_ps = ppA.tile([P, P], bf16, tag="tr_ps")
                        nc.tensor.transpose(
                            xgT_ps[:], xg_all[:, t, k * P:(k + 1) * P], ident_bf[:]
                        )
                        xgT = sb.tile([P, P], bf16, tag="xgT")
                        nc.scalar.activation(
                            out=xgT[:], in_=xgT_ps[:], func=Act.Copy
                        )
                        nc.tensor.matmul(
                            xwps[:], lhsT=xgT[:], rhs=wc_t[:, k, :],
                            start=(k == 0), stop=(k == 1),
                        )
                    ub = gp.tile([P, HID], bf16, tag="ub")
                    nc.vector.tensor_scalar(
                        out=ub[:], in0=xwps[:], scalar1=dinv_t[:, t:t + 1],
                        scalar2=None, op0=mybir.AluOpType.mult,
                    )
                    nc.sync.dma_start(out=ag_in[t * P:(t + 1) * P, :], in_=ub[:])

            # ---- phase D: AllGather u across cores ----
            nc.gpsimd.collective_compute(
                "AllGather",
                mybir.AluOpType.bypass,
                replica_groups=[list(range(NCORES))],
                ins=[ag_in[:]],
                outs=[ag_out[:]],
            )

            # ---- phase E: GCN via slot-major gathers + classifier ----
            goff = 0
            for t in range(TPC):
                dlo, dhi = D2lo[t], D2hi[t]
                D = dlo + dhi
                w = 8 * D
                idx_t = gcn_ix[:, goff:goff + w]
                goff += w
                Gu = gu.tile([P, D, HID], bf16, tag="Gu")
                if dlo > 0:
                    nc.gpsimd.dma_gather(
                        out_ap=Gu[:, 0:dlo, :],
                        in_ap=ag_out[:, :],
                        idxs_ap=idx_t[:, 0:8 * dlo],
                        num_idxs=P * dlo,
                        num_idxs_reg=P * dlo,
                        elem_size=HID,
                        single_packet=False,
                        queue_num=qrr[0] % 4,
                    )
                    qrr[0] += 1
                if dhi > 0:
                    nc.gpsimd.dma_gather(
                        out_ap=Gu[:, dlo:D, :],
                        in_ap=ag_out[SPLIT:, :],
                        idxs_ap=idx_t[:, 8 * dlo:w],
                        num_idxs=P * dhi,
                        num_idxs_reg=P * dhi,
                        elem_size=HID,
                        single_packet=False,
                        queue_num=qrr[0] % 4,
                    )
                    qrr[0] += 1
                # tree-reduce over D slots in place (bf16), tail in f32
                cur = D
                while cur > 6:
                    h2 = cur // 2
                    nc.vector.tensor_tensor(
                        out=Gu[:, 0:h2, :], in0=Gu[:, 0:h2, :],
                        in1=Gu[:, h2:2 * h2, :], op=mybir.AluOpType.add,
                    )
                    if cur % 2:
                        nc.vector.tensor_tensor(
                            out=Gu[:, 0, :], in0=Gu[:, 0, :],
                            in1=Gu[:, 2 * h2, :], op=mybir.AluOpType.add,
                        )
                    cur = h2
                uacc = sp.tile([P, HID], f32, tag="uaccF")
                if cur == 1:
                    nc.vector.tensor_copy(out=uacc[:], in_=Gu[:, 0, :])
                else:
                    nc.vector.tensor_tensor(
                        out=uacc[:], in0=Gu[:, 0, :], in1=Gu[:, 1, :],
                        op=mybir.AluOpType.add,
                    )
                    for j in range(2, cur):
                        nc.vector.tensor_tensor(
                            out=uacc[:], in0=uacc[:], in1=Gu[:, j, :],
                            op=mybir.AluOpType.add,
                        )
                # xc = relu(dinv * sum + bc), bf16 for the transpose
                xc = gu.tile([P, HID], bf16, tag="xc")
                nc.vector.tensor_scalar(
                    out=uacc[:], in0=uacc[:], scalar1=dinv2_t[:, t:t + 1],
                    scalar2=None, op0=mybir.AluOpType.mult,
                )
                nc.vector.tensor_tensor(
                    out=uacc[:], in0=uacc[:], in1=bc_t[:], op=mybir.AluOpType.add
                )
                nc.vector.tensor_scalar(
                    out=xc[:], in0=uacc[:], scalar1=0.0, scalar2=None,
                    op0=mybir.AluOpType.max,
                )
                # classifier: transpose xc, then bf16 matmul
                xcT_ps = ppA.tile([P, P], bf16, tag="tr_ps")
                nc.tensor.transpose(xcT_ps[:], xc[:], ident_bf[:])
                xcT = sb.tile([P, P], bf16, tag="xcT")
                nc.scalar.activation(out=xcT[:], in_=xcT_ps[:], func=Act.Copy)
                lps = ppB.tile([P, NCLASS], f32, tag="mm_psB")
                nc.tensor.matmul(lps[:], lhsT=xcT[:], rhs=wlb_t[:],
                                 start=True, stop=True)
                nc.vector.tensor_tensor(
                    out=lg_all[:, t, :], in0=lps[:], in1=bl_t[:],
                    op=mybir.AluOpType.add,
                )

            # ---- bulk log_softmax over all tiles ----
            nmx = sp.tile([P, TPC], f32, tag="nmx")
            nc.vector.tensor_reduce(
                out=nmx[:], in_=lg_all[:], axis=mybir.AxisListType.X,
                op=mybir.AluOpType.max, negate=True,
            )
            nc.vector.tensor_tensor(
                out=lg_all[:], in0=lg_all[:],
                in1=nmx[:, :, None].to_broadcast([P, TPC, NCLASS]),
                op=mybir.AluOpType.add,
            )
            exl = sp.tile([P, TPC, NCLASS], f32, tag="exl")
            nc.scalar.activation(out=exl[:], in_=lg_all[:], func=Act.Exp)
            sume = sp.tile([P, TPC], f32, tag="sume")
            nc.vector.tensor_reduce(
                out=sume[:], in_=exl[:], axis=mybir.AxisListType.X,
                op=mybir.AluOpType.add,
            )
            lns = sp.tile([P, TPC], f32, tag="lns")
            nc.scalar.activation(out=lns[:], in_=sume[:], func=Act.Ln)
            nc.vector.tensor_tensor(
                out=lg_all[:], in0=lg_all[:],
                in1=lns[:, :, None].to_broadcast([P, TPC, NCLASS]),
                op=mybir.AluOpType.subtract,
            )
            nc.sync.dma_start(
                out=out.rearrange("(t p) c -> p t c", p=P), in_=lg_all[:]
            )

    nc.compile()
    return nc


def _prepare(inputs):
    x = np.asarray(inputs["x"], np.float32)
    Wg = np.asarray(inputs["Wg"], np.float32)
    att_src = np.asarray(inputs["att_src"], np.float32)
    att_dst = np.asarray(inputs["att_dst"], np.float32)
    bg = np.asarray(inputs["bg"], np.float32)
    Wc = np.asarray(inputs["Wc"], np.float32)
    bc = np.asarray(inputs["bc"], np.float32)
    Wl = np.asarray(inputs["Wl"], np.float32)
    bl = np.asarray(inputs["bl"], np.float32)
    edge_index = np.asarray(inputs["edge_index"])

    st = _build_structures(edge_index)

    # fold attention vectors into the feature matmul: a_s = x @ (Wg @ As)
    As = np.zeros((HC, H), np.float32)
    Ad = np.zeros((HC, H), np.float32)
    for h in range(H):
        As[h * C:(h + 1) * C, h] = att_src[h]
        Ad[h * C:(h + 1) * C, h] = att_dst[h]
    wg_aug = np.concatenate(
        [Wg, Wg @ As, np.zeros((F_IN, TABW - HC - H), np.float32)], axis=1
    )  # [128, 384], zero-padded so phase A initializes full table rows
    wg_ad = Wg @ Ad                                 # [128, 4]

    x_padT = np.zeros((F_IN, NPAD), np.float32)
    x_padT[:, :N] = x.T

    bf = ml_dtypes.bfloat16
    in_maps = []
    for c in range(NCORES):
        xpT = np.zeros((F_IN, S), np.float32)
        xpT[:, :NPC] = x[st["perm"][c]].T
        dv = np.zeros((P, TPC), np.float32)
        dvp = np.zeros(S, np.float32)
        dvp[:NPC] = st["dinv"][st["perm"][c]]
        dv[:, :] = dvp.reshape(TPC, P).T
        dv2 = np.zeros((P, TPC), np.float32)
        dvp2 = np.zeros(S, np.float32)
        dvp2[:NPC] = st["dinv"][st["perm2"][c]]
        dv2[:, :] = dvp2.reshape(TPC, P).T
        in_maps.append({
            "x_padT": x_padT.astype(bf),
            "x_permT": xpT.astype(bf),
            "dinv_pt": dv,
            "dinv2_pt": dv2,
            "gat_idx": st["gat_idx"][c],
            "gcn_idx": st["gcn_idx"][c],
            "wg_aug": wg_aug.astype(bf),
            "wg_ad": wg_ad.astype(bf),
            "wc": Wc.astype(bf),
            "wl": Wl.astype(bf),
            "bg_b": np.tile(bg[None, :], (P, 1)),
            "bc_b": np.tile(bc[None, :], (P, 1)),
            "bl_b": np.tile(bl[None, :], (P, 1)),
            "ident_bf": np.eye(P, dtype=bf),
            "ident_f": np.eye(P, dtype=np.float32),
        })
    return st, in_maps


def _run(inputs, trace=False, trace_kwargs=None):
    st, in_maps = _prepare(inputs)
    nc = _build_kernel(
        st["Dlo"], st["Dhi"], st["D2lo"], st["D2hi"],
        st["gat_idx"][0].shape[1], st["gcn_idx"][0].shape[1],
    )
    res = run_bass_kernel_spmd(
        nc, in_maps, list(range(NCORES)), trace=trace, **(trace_kwargs or {})
    )
    out = np.empty((N, NCLASS), np.float32)
    for c in range(NCORES):
        out[st["perm2"][c]] = res.results[c]["out"][:NPC]
    return out, res


def kernel(**inputs) -> np.ndarray:
    out, _ = _run(inputs, trace=False)
    return out


# revision 16
# speedup vs baseline: 1.0698x; 1.0698x over previous
"""GAT + GCN + classifier over a COO graph, distributed over 8 TRN2 NeuronCores.

v2 strategy (descriptor- and instruction-count driven):
  - Nodes dealt to 8 cores by (d_lo1, d_hi1) lexsort round-robin (balances
    both the per-core edge count and the per-tile degree profiles).
  - Phase A: every core builds the full gather table T[n] = [h(n) | a_s(n)]
    (bf16, 768B rows) from x^T tiles (host-pretransposed, so no on-device
    transposes); PSUM->SBUF casts alternate DVE/ACT, paired 2 tiles/instr.
  - GAT phase: per dst-tile slot-major dma_gather (lo/hi int16 halves),
    softmax fused into ~7 wide instructions (ACT Lrelu + Exp, DVE broadcast
    adds + reduces), weighted sum via broadcast multiply + pairwise tree.
    ELU + bias deferred to one bulk pass per 8-tile group.
  - GCN phase: u = dinv*(xg @ Wc) AllGathered, then TRANSPOSED dma_gather
    (lane-major columns) -> per-lane tensor_reduce gives xc^T directly ->
    ACT relu(+bc) -> classifier matmul without any transposes. dst nodes
    re-sorted per core by GCN degree profile (perm2) to cut slot padding.
  - log_softmax bulk at the end; host de-permutes rows (perm2 order).
"""
import sys

sys.path.insert(0, "/opt/trn_rl_repo")

import numpy as np
import ml_dtypes

import concourse.bass as bass
import concourse.bacc as bacc
import concourse.mybir as mybir
import concourse.tile as tile
from concourse.bass_utils import run_bass_kernel_spmd

# problem constants (hardcoded per contract)
N = 50000
E = 800000
F_IN = 128
H = 4
C = 64
HC = H * C          # 256
HID = 128
NCLASS = 10
NEG = 0.2

NCORES = 8
P = 128
NPC = N // NCORES   # 6250 nodes per core
TPC = 49            # tiles per core (49*128 = 6272 >= 6250)
S = TPC * P         # 6272 padded slots per core
SPLIT = 32768       # int16 gather index range per table half
XT = 391            # x tiles for table build (391*128 = 50048)
NPAD = XT * P       # 50048
NT_ROWS = 1 + NPAD + 1   # gather table rows: [dummy | nodes (+pad) | hi dummy]
HI_DUMMY = NT_ROWS - 1   # 50049
TABW = 384          # bf16 table row: 0:256 h, 256:260 a_s, 260:384 junk pad
ASD_NEG = -10000.0  # a_s marker for dummy rows (drives softmax weight to ~0)
NU_ROWS = NCORES * S     # 50176 u-table rows
U_LO_DUMMY = NPC         # row 6250 (core0 pad slot -> always zero)
U_HI_DUMMY = 7 * S + NPC # row 50154 (core7 pad slot)
GRP = 8             # GAT tiles per ELU/phase-C group

f32 = mybir.dt.float32
bf16 = mybir.dt.bfloat16
i16 = mybir.dt.int16


def _build_structures(edge_index):
    src = np.asarray(edge_index[0], dtype=np.int64)
    dst = np.asarray(edge_index[1], dtype=np.int64)
    src = np.concatenate([src, np.arange(N, dtype=np.int64)])
    dst = np.concatenate([dst, np.arange(N, dtype=np.int64)])
    deg = np.bincount(dst, minlength=N).astype(np.int64)
    dinv = (1.0 / np.sqrt(deg)).astype(np.float32)

    indptr = np.zeros(N + 1, np.int64)
    np.cumsum(deg, out=indptr[1:])

    # --- GAT split: table row of node n follows the interleaved phase-A
    #     write pattern (chunk i4 of 4 x-tiles: row = 1 + i4*512 + p*n_t + k)
    row_of = np.empty(NPAD, np.int64)
    for i4 in range((XT + 3) // 4):
        n_t = min(4, XT - i4 * 4)
        for k in range(n_t):
            cols = (i4 * 4 + k) * P + np.arange(P)
            row_of[cols] = 1 + i4 * 4 * P + np.arange(P) * n_t + k
    rr = row_of[src]
    hi1 = rr >= SPLIT
    d_hi1 = np.bincount(dst[hi1], minlength=N).astype(np.int64)
    d_lo1 = deg - d_hi1
    order1 = np.lexsort((hi1, dst))
    adj1 = rr[order1]  # table rows, grouped by dst, lo sources first

    # --- node -> core deal by (d_lo1, d_hi1) lexsort (matches tile profiles
    #     across cores), then per-core sort by the same keys ---
    key_order = np.lexsort((d_hi1, d_lo1))
    perm = np.empty((NCORES, NPC), np.int64)
    for c in range(NCORES):
        nodes = key_order[c::NCORES]
        k = np.lexsort((d_hi1[nodes], d_lo1[nodes]))
        perm[c] = nodes[k]
    pos = np.empty(N, np.int64)
    for c in range(NCORES):
        pos[perm[c]] = c * S + np.arange(NPC)

    # --- GCN split: u-table row of node n is pos[n] ---
    ps = pos[src]
    hi2 = ps >= SPLIT
    d_hi2 = np.bincount(dst[hi2], minlength=N).astype(np.int64)
    d_lo2 = deg - d_hi2
    order2 = np.lexsort((hi2, dst))
    adj2 = ps[order2]  # u-table positions, grouped by dst, lo first

    # --- per-core GCN re-sort (perm2): same node set, ordered by GCN keys ---
    perm2 = np.empty((NCORES, NPC), np.int64)
    for c in range(NCORES):
        nodes = perm[c]
        k = np.lexsort((d_hi2[nodes], d_lo2[nodes]))
        perm2[c] = nodes[k]

    # --- common (max across cores) per-tile slot profiles ---
    def tile_prof(dvals, pm):
        m = np.zeros((NCORES, S), np.int64)
        for c in range(NCORES):
            m[c, :NPC] = dvals[pm[c]]
        return m.reshape(NCORES, TPC, P).max(axis=(0, 2))

    Dlo = tile_prof(d_lo1, perm)
    Dhi = tile_prof(d_hi1, perm)
    D2lo = tile_prof(d_lo2, perm2)
    D2hi = tile_prof(d_hi2, perm2)

    def block(nodes, Dt, dcount, base, adj, shift, dummy, lane_major):
        """Padded [Dt*128] int index block for one tile."""
        if Dt == 0:
            return np.zeros(0, np.int64)
        nv = np.maximum(nodes, 0)
        cnt = np.where(nodes >= 0, dcount[nv], 0)
        sl = np.arange(Dt)
        ei = base[:, None] + sl[None, :]
        valid = sl[None, :] < cnt[:, None]
        vals = np.where(valid, adj[np.where(valid, ei, 0)] + shift, dummy)
        if lane_major:
            return vals.reshape(-1)       # position = lane*Dt + slot
        return vals.T.reshape(-1)         # position = slot*128 + lane

    def wrap16(flat):
        # position i -> [i % 16, i // 16], replicated to 128 partitions
        arr = flat.reshape(-1, 16).T
        return np.tile(arr, (8, 1))

    gat_idx = []
    gcn_idx = []
    for c in range(NCORES):
        nodes_pad = np.full(S, -1, np.int64)
        nodes_pad[:NPC] = perm[c]
        nodes_pad2 = np.full(S, -1, np.int64)
        nodes_pad2[:NPC] = perm2[c]
        cols1 = []
        cols2 = []
        for t in range(TPC):
            nodes = nodes_pad[t * P:(t + 1) * P]
            nv = np.maximum(nodes, 0)
            b_lo1 = indptr[nv]
            b_hi1 = indptr[nv] + d_lo1[nv]
            lo = block(nodes, Dlo[t], d_lo1, b_lo1, adj1, 0, 0, False)
            hi = block(nodes, Dhi[t], d_hi1, b_hi1, adj1, -SPLIT,
                       HI_DUMMY - SPLIT, False)
            assert lo.size == 0 or (0 <= lo.min() and lo.max() < SPLIT)
            assert hi.size == 0 or (0 <= hi.min() and hi.max() <= HI_DUMMY - SPLIT)
            cols1.append(wrap16(lo))
            cols1.append(wrap16(hi))

            nodes2 = nodes_pad2[t * P:(t + 1) * P]
            nv2 = np.maximum(nodes2, 0)
            b_lo2 = indptr[nv2]
            b_hi2 = indptr[nv2] + d_lo2[nv2]
            lo2 = block(nodes2, D2lo[t], d_lo2, b_lo2, adj2, 0, U_LO_DUMMY, False)
            hi2b = block(nodes2, D2hi[t], d_hi2, b_hi2, adj2, -SPLIT,
                         U_HI_DUMMY - SPLIT, False)
            assert lo2.size == 0 or (0 <= lo2.min() and lo2.max() < SPLIT)
            assert hi2b.size == 0 or (0 <= hi2b.min() and hi2b.max() < SPLIT)
            cols2.append(wrap16(lo2))
            cols2.append(wrap16(hi2b))
        gat_idx.append(np.concatenate(cols1, axis=1).astype(np.int16))
        gcn_idx.append(np.concatenate(cols2, axis=1).astype(np.int16))

    return dict(
        dinv=dinv, perm=perm, perm2=perm2,
        Dlo=Dlo.tolist(), Dhi=Dhi.tolist(),
        D2lo=D2lo.tolist(), D2hi=D2hi.tolist(),
        gat_idx=gat_idx, gcn_idx=gcn_idx,
    )


def _build_kernel(Dlo, Dhi, D2lo, D2hi, gat_cols, gcn_cols):
    nc = bacc.Bacc(None, num_devices=NCORES, num_swdge_queues=4)

    x_padT = nc.declare_dram_parameter("x_padT", [F_IN, NPAD], bf16, isOutput=False)
    x_permT = nc.declare_dram_parameter("x_permT", [F_IN, S], bf16, isOutput=False)
    dinv_pt = nc.declare_dram_parameter("dinv_pt", [P, TPC], f32, isOutput=False)
    dinv2_pt = nc.declare_dram_parameter("dinv2_pt", [P, TPC], f32, isOutput=False)
    gat_idx = nc.declare_dram_parameter("gat_idx", [P, gat_cols], i16, isOutput=False)
    gcn_idx = nc.declare_dram_parameter("gcn_idx", [P, gcn_cols], i16, isOutput=False)
    wg_aug = nc.declare_dram_parameter("wg_aug", [F_IN, TABW], bf16, isOutput=False)
    wg_ad = nc.declare_dram_parameter("wg_ad", [F_IN, H], bf16, isOutput=False)
    wc = nc.declare_dram_parameter("wc", [HC, HID], bf16, isOutput=False)
    wl = nc.declare_dram_parameter("wl", [HID, NCLASS], bf16, isOutput=False)
    bg_b = nc.declare_dram_parameter("bg_b", [P, HC], f32, isOutput=False)
    bc_b = nc.declare_dram_parameter("bc_b", [P, HID], f32, isOutput=False)
    bl_b = nc.declare_dram_parameter("bl_b", [P, NCLASS], f32, isOutput=False)
    ident_bf_in = nc.declare_dram_parameter("ident_bf", [P, P], bf16, isOutput=False)
    ident_f_in = nc.declare_dram_parameter("ident_f", [P, P], f32, isOutput=False)
    out = nc.declare_dram_parameter("out", [S, NCLASS], f32, isOutput=True)

    h_table = nc.dram_tensor("h_table", [NT_ROWS, TABW], bf16)
    ag_in = nc.dram_tensor("ag_in", [S, HID], bf16)
    ag_out = nc.dram_tensor("ag_out", [NU_ROWS, HID], bf16, addr_space="Shared")

    AW = TABW
    Act = mybir.ActivationFunctionType

    with tile.TileContext(nc) as tc:
        with (
            tc.tile_pool(name="const", bufs=1) as cpool,
            tc.tile_pool(name="sbuf", bufs=3) as sb,
            tc.tile_pool(name="gat", bufs=2) as gp,
            tc.tile_pool(name="scratch", bufs=1) as sp,
            tc.tile_pool(name="softmax", bufs=3) as spE,
            tc.tile_pool(name="gut", bufs=3) as gu,
            tc.tile_pool(name="psA", bufs=2, space="PSUM") as ppA,
            tc.tile_pool(name="psB", bufs=2, space="PSUM") as ppB,
        ):
            # ---- resident constants ----
            ident_bf = cpool.tile([P, P], bf16)
            nc.sync.dma_start(out=ident_bf[:], in_=ident_bf_in[:])
            wga_t = cpool.tile([F_IN, AW], bf16)
            nc.sync.dma_start(out=wga_t[:], in_=wg_aug[:])
            wgad_t = cpool.tile([F_IN, H], bf16)
            nc.sync.dma_start(out=wgad_t[:], in_=wg_ad[:])
            wc_t = cpool.tile([P, 2, HID], bf16)
            nc.sync.dma_start(out=wc_t[:], in_=wc.rearrange("(k p) n -> p k n", p=P))
            wlb_t = cpool.tile([HID, NCLASS], bf16)
            nc.sync.dma_start(out=wlb_t[:], in_=wl[:])
            bg_t = cpool.tile([P, HC], f32)
            nc.sync.dma_start(out=bg_t[:], in_=bg_b[:])
            bc_t = cpool.tile([P, HID], f32)
            nc.sync.dma_start(out=bc_t[:], in_=bc_b[:])
            ident_f = cpool.tile([P, P], f32)
            nc.sync.dma_start(out=ident_f[:], in_=ident_f_in[:])
            bl_t = cpool.tile([P, NCLASS], f32)
            nc.sync.dma_start(out=bl_t[:], in_=bl_b[:])
            dinv_t = cpool.tile([P, TPC], f32)
            nc.sync.dma_start(out=dinv_t[:], in_=dinv_pt[:])
            dinv2_t = cpool.tile([P, TPC], f32)
            nc.sync.dma_start(out=dinv2_t[:], in_=dinv2_pt[:])
            ad_all = cpool.tile([P, TPC, H], f32)
            xg_all = cpool.tile([P, TPC, HC], bf16)
            lg_all = cpool.tile([P, TPC, NCLASS], f32)

            # ---- dummy table rows ----
            dum = cpool.tile([1, TABW], bf16)
            nc.vector.memset(dum[:], 0.0)
            nc.vector.memset(dum[:, HC:HC + H], ASD_NEG)
            nc.sync.dma_start(out=h_table[0:1, :], in_=dum[:])
            nc.sync.dma_start(out=h_table[HI_DUMMY:HI_DUMMY + 1, :], in_=dum[:])

            # ---- phase A: build gather table rows 1..NPAD ----
            # 4 tiles per chunk: one 1KB-contiguous x read and one 3KB-
            # contiguous interleaved table write (row of node = host pi map).
            for i4 in range((XT + 3) // 4):
                n_t = min(4, XT - i4 * 4)
                xq = sb.tile([P, 4 * P], bf16, tag="xq")
                nc.sync.dma_start(
                    out=xq[:, 0:n_t * P],
                    in_=x_padT[:, i4 * 4 * P:(i4 * 4 + n_t) * P],
                )
                hbf = sb.tile([P, 4, AW], bf16, tag="hbf")
                for k2 in range((n_t + 1) // 2):
                    n_p = min(2, n_t - k2 * 2)
                    hps = ppA.tile([P, 2, 512], f32, tag="mm_psA")
                    for k in range(n_p):
                        nc.tensor.matmul(
                            hps[:, k, 0:AW],
                            lhsT=xq[:, (k2 * 2 + k) * P:(k2 * 2 + k + 1) * P],
                            rhs=wga_t[:], start=True, stop=True,
                        )
                    src_ap = hps[:, 0:n_p, 0:AW]
                    dst_ap = hbf[:, k2 * 2:k2 * 2 + n_p, :]
                    if k2 % 2 == 0:
                        nc.vector.tensor_copy(out=dst_ap, in_=src_ap)
                    else:
                        nc.scalar.activation(out=dst_ap, in_=src_ap, func=Act.Copy)
                nc.sync.dma_start(
                    out=h_table[
                        1 + i4 * 4 * P:1 + (i4 * 4 + n_t) * P, :
                    ].rearrange("(p k) w -> p k w", k=n_t),
                    in_=hbf[:, 0:n_t, :],
                )

            # ---- phase A2: per-tile a_d for this core's own nodes ----
            for t in range(TPC):
                xT = sb.tile([P, P], bf16, tag="xT")
                nc.sync.dma_start(out=xT[:], in_=x_permT[:, t * P:(t + 1) * P])
                adps = ppB.tile([P, H], f32, tag="mm_psB")
                nc.tensor.matmul(adps[:], lhsT=xT[:], rhs=wgad_t[:],
                                 start=True, stop=True)
                nc.vector.tensor_copy(out=ad_all[:, t, :], in_=adps[:])

            # ---- phase B: GAT per tile; ELU + phase C per group of 8 ----
            qrr = [0]  # swdge queue round-robin
            goff = 0
            for g0 in range(0, TPC, GRP):
                g1 = min(g0 + GRP, TPC)
                for t in range(g0, g1):
                    dlo, dhi = Dlo[t], Dhi[t]
                    D = dlo + dhi
                    w = 8 * D
                    idx_t = gp.tile([P, w], i16, tag="gidx")
                    nc.sync.dma_start(out=idx_t[:], in_=gat_idx[:, goff:goff + w])
                    goff += w
                    G = gp.tile([P, D, TABW], bf16, tag="G")
                    if dlo > 0:
                        nc.gpsimd.dma_gather(
                            out_ap=G[:, 0:dlo, :],
                            in_ap=h_table[:, :],
                            idxs_ap=idx_t[:, 0:8 * dlo],
                            num_idxs=P * dlo,
                            num_idxs_reg=P * dlo,
                            elem_size=TABW,
                            single_packet=False,
                            queue_num=qrr[0] % 4,
                        )
                        qrr[0] += 1
                    if dhi > 0:
                        nc.gpsimd.dma_gather(
                            out_ap=G[:, dlo:D, :],
                            in_ap=h_table[SPLIT:, :],
                            idxs_ap=idx_t[:, 8 * dlo:w],
                            num_idxs=P * dhi,
                            num_idxs_reg=P * dhi,
                            elem_size=TABW,
                            single_packet=False,
                            queue_num=qrr[0] % 4,
                        )
                        qrr[0] += 1
                    # e = leaky_relu(a_s[src] + a_d[dst])   [P, H, D]
                    e = spE.tile([P, H, D], f32, tag="e")
                    nc.vector.tensor_tensor(
                        out=e[:],
                        in0=G[:, :, HC:HC + H].rearrange("p d h -> p h d"),
                        in1=ad_all[:, t, :][:, :, None].to_broadcast([P, H, D]),
                        op=mybir.AluOpType.add,
                    )
                    e2 = spE.tile([P, H, D], f32, tag="e2")
                    nc.vector.tensor_scalar(
                        out=e2[:], in0=e[:], scalar1=NEG, scalar2=None,
                        op0=mybir.AluOpType.mult,
                    )
                    nc.vector.tensor_tensor(
                        out=e[:], in0=e[:], in1=e2[:], op=mybir.AluOpType.max
                    )
                    negm = spE.tile([P, H], f32, tag="negm")
                    nc.vector.tensor_reduce(
                        out=negm[:], in_=e[:], axis=mybir.AxisListType.X,
                        op=mybir.AluOpType.max, negate=True,
                    )
                    ex = spE.tile([P, H, D], f32, tag="ex")
                    nc.vector.tensor_tensor(
                        out=ex[:], in0=e[:],
                        in1=negm[:, :, None].to_broadcast([P, H, D]),
                        op=mybir.AluOpType.add,
                    )
                    nc.scalar.activation(out=ex[:], in_=ex[:], func=Act.Exp)
                    den = spE.tile([P, H], f32, tag="den")
                    nc.vector.tensor_reduce(
                        out=den[:], in_=ex[:], axis=mybir.AxisListType.X,
                        op=mybir.AluOpType.add,
                    )
                    rden = spE.tile([P, H], f32, tag="rden")
                    nc.vector.reciprocal(rden[:], den[:])
                    # prod[p, d, h, c] = h_gathered * ex  (bf16)
                    prod = sp.tile([P, D, HC], bf16, tag="prod")
                    g_h = G[:, :, 0:HC].rearrange("p d (h c) -> p d h c", h=H)
                    ex_b = ex.rearrange("p h d -> p d h")[:, :, :, None].to_broadcast(
                        [P, D, H, C]
                    )
                    nc.vector.tensor_tensor(
                        out=prod.rearrange("p d (h c) -> p d h c", h=H),
                        in0=g_h, in1=ex_b, op=mybir.AluOpType.mult,
                    )
                    # tree-reduce over D slots, in place on prod (bf16
                    # until width <= 6, then one f32 level)
                    cur = D
                    while cur > 6:
                        h2 = cur // 2
                        nc.vector.tensor_tensor(
                            out=prod[:, 0:h2, :], in0=prod[:, 0:h2, :],
                            in1=prod[:, h2:2 * h2, :], op=mybir.AluOpType.add,
                        )
                        if cur % 2:
                            nc.vector.tensor_tensor(
                                out=prod[:, 0, :], in0=prod[:, 0, :],
                                in1=prod[:, 2 * h2, :], op=mybir.AluOpType.add,
                            )
                        cur = h2
                    acc = sp.tile([P, HC], f32, tag="accF")
                    if cur == 1:
                        nc.vector.tensor_copy(out=acc[:], in_=prod[:, 0, :])
                    else:
                        nc.vector.tensor_tensor(
                            out=acc[:], in0=prod[:, 0, :], in1=prod[:, 1, :],
                            op=mybir.AluOpType.add,
                        )
                        for j in range(2, cur):
                            nc.vector.tensor_tensor(
                                out=acc[:], in0=acc[:], in1=prod[:, j, :],
                                op=mybir.AluOpType.add,
                            )
                    # xg_raw = acc / den  -> bulk buffer (bf16)
                    nc.vector.tensor_tensor(
                        out=xg_all[:, t, :].rearrange("p (h c) -> p h c", h=H),
                        in0=acc.rearrange("p (h c) -> p h c", h=H),
                        in1=rden[:, :, None].to_broadcast([P, H, C]),
                        op=mybir.AluOpType.mult,
                    )

                # ---- group ELU: xg = elu(xg_raw + bg) in-place (bf16) ----
                ng = g1 - g0
                xs = xg_all[:, g0:g1, :]
                nc.vector.tensor_tensor(
                    out=xs, in0=xs,
                    in1=bg_t[:, None, :].to_broadcast([P, ng, HC]),
                    op=mybir.AluOpType.add,
                )
                tneg = sp.tile([P, GRP * HC], f32, tag="tneg")
                tn = tneg[:, 0:ng * HC].rearrange("p (g c) -> p g c", g=ng)
                nc.vector.tensor_scalar(
                    out=tn, in0=xs, scalar1=0.0, scalar2=None,
                    op0=mybir.AluOpType.min,
                )
                nc.scalar.activation(out=tn, in_=tn, func=Act.Exp)
                nc.vector.tensor_scalar(
                    out=xs, in0=xs, scalar1=0.0, scalar2=None,
                    op0=mybir.AluOpType.max,
                )
                nc.vector.tensor_tensor(
                    out=xs, in0=xs, in1=tn, op=mybir.AluOpType.add,
                )
                nc.vector.tensor_scalar(
                    out=xs, in0=xs, scalar1=1.0, scalar2=None,
                    op0=mybir.AluOpType.subtract,
                )

                # ---- phase C for the group: u = dinv * (xg @ Wc) ----
                for t in range(g0, g1):
                    xwps = ppB.tile([P, HID], f32, tag="mm_psB")
                    for k in range(2):
                        xgT_ps = ppA.tile([P, P], bf16, tag="tr_ps")
                        nc.tensor.transpose(
                            xgT_ps[:], xg_all[:, t, k * P:(k + 1) * P], ident_bf[:]
                        )
                        xgT = sb.tile([P, P], bf16, tag="xgT")
                        nc.scalar.activation(
                            out=xgT[:], in_=xgT_ps[:], func=Act.Copy
                        )
                        nc.tensor.matmul(
                            xwps[:], lhsT=xgT[:], rhs=wc_t[:, k, :],
                            start=(k == 0), stop=(k == 1),
                        )
                    ub = gp.tile([P, HID], bf16, tag="ub")
                    nc.vector.tensor_scalar(
                        out=ub[:], in0=xwps[:], scalar1=dinv_t[:, t:t + 1],
                        scalar2=None, op0=mybir.AluOpType.mult,
                    )
                    nc.sync.dma_start(out=ag_in[t * P:(t + 1) * P, :], in_=ub[:])

            # ---- phase D: AllGather u across cores ----
            nc.gpsimd.collective_compute(
                "AllGather",
                mybir.AluOpType.bypass,
                replica_groups=[list(range(NCORES))],
                ins=[ag_in[:]],
                outs=[ag_out[:]],
            )

            # ---- phase E: GCN via slot-major gathers + classifier ----
            goff = 0
            for t in range(TPC):
                dlo, dhi = D2lo[t], D2hi[t]
                D = dlo + dhi
                w = 8 * D
                idx_t = gu.tile([P, w], i16, tag="gidx2")
                nc.sync.dma_start(out=idx_t[:], in_=gcn_idx[:, goff:goff + w])
                goff += w
                Gu = gu.tile([P, D, HID], bf16, tag="Gu")
                if dlo > 0:
                    nc.gpsimd.dma_gather(
                        out_ap=Gu[:, 0:dlo, :],
                        in_ap=ag_out[:, :],
                        idxs_ap=idx_t[:, 0:8 * dlo],
                        num_idxs=P * dlo,
                        num_idxs_reg=P * dlo,
                        elem_size=HID,
                        single_packet=False,
                        queue_num=qrr[0] % 4,
                    )
                    qrr[0] += 1
                if dhi > 0:
                    nc.gpsimd.dma_gather(
                        out_ap=Gu[:, dlo:D, :],
                        in_ap=ag_out[SPLIT:, :],
                        idxs_ap=idx_t[:, 8 * dlo:w],
                        num_idxs=P * dhi,
                        num_idxs_reg=P * dhi,
                        elem_size=HID,
                        single_packet=False,
                        queue_num=qrr[0] % 4,
                    )
                    qrr[0] += 1
                # tree-reduce over D slots in place (bf16), tail in f32
                cur = D
                while cur > 6:
                    h2 = cur // 2
                    nc.vector.tensor_tensor(
                        out=Gu[:, 0:h2, :], in0=Gu[:, 0:h2, :],
                        in1=Gu[:, h2:2 * h2, :], op=mybir.AluOpType.add,
                    )
                    if cur % 2:
                        nc.vector.tensor_tensor(
                            out=Gu[:, 0, :], in0=Gu[:, 0, :],
                            in1=Gu[:, 2 * h2, :], op=mybir.AluOpType.add,
                        )
                    cur = h2
                uacc = sp.tile([P, HID], f32, tag="uaccF")
                if cur == 1:
                    nc.vector.tensor_copy(out=uacc[:], in_=Gu[:, 0, :])
                else:
                    nc.vector.tensor_tensor(
                        out=uacc[:], in0=Gu[:, 0, :], in1=Gu[:, 1, :],
                        op=mybir.AluOpType.add,
                    )
                    for j in range(2, cur):
                        nc.vector.tensor_tensor(
                            out=uacc[:], in0=uacc[:], in1=Gu[:, j, :],
                            op=mybir.AluOpType.add,
                        )
                # xc = relu(dinv * sum + bc), bf16 for the transpose
                xc = gu.tile([P, HID], bf16, tag="xc")
                nc.vector.tensor_scalar(
                    out=uacc[:], in0=uacc[:], scalar1=dinv2_t[:, t:t + 1],
                    scalar2=None, op0=mybir.AluOpType.mult,
                )
                nc.vector.tensor_tensor(
                    out=uacc[:], in0=uacc[:], in1=bc_t[:], op=mybir.AluOpType.add
                )
                nc.vector.tensor_scalar(
                    out=xc[:], in0=uacc[:], scalar1=0.0, scalar2=None,
                    op0=mybir.AluOpType.max,
                )
                # classifier: transpose xc, then bf16 matmul
                xcT_ps = ppA.tile([P, P], bf16, tag="tr_ps")
                nc.tensor.transpose(xcT_ps[:], xc[:], ident_bf[:])
                xcT = sb.tile([P, P], bf16, tag="xcT")
                nc.scalar.activation(out=xcT[:], in_=xcT_ps[:], func=Act.Copy)
                lps = ppB.tile([P, NCLASS], f32, tag="mm_psB")
                nc.tensor.matmul(lps[:], lhsT=xcT[:], rhs=wlb_t[:],
                                 start=True, stop=True)
                nc.vector.tensor_tensor(
                    out=lg_all[:, t, :], in0=lps[:], in1=bl_t[:],
                    op=mybir.AluOpType.add,
                )

            # ---- bulk log_softmax over all tiles ----
            nmx = sp.tile([P, TPC], f32, tag="nmx")
            nc.vector.tensor_reduce(
                out=nmx[:], in_=lg_all[:], axis=mybir.AxisListType.X,
                op=mybir.AluOpType.max, negate=True,
            )
            nc.vector.tensor_tensor(
                out=lg_all[:], in0=lg_all[:],
                in1=nmx[:, :, None].to_broadcast([P, TPC, NCLASS]),
                op=mybir.AluOpType.add,
            )
            exl = sp.tile([P, TPC, NCLASS], f32, tag="exl")
            nc.scalar.activation(out=exl[:], in_=lg_all[:], func=Act.Exp)
            sume = sp.tile([P, TPC], f32, tag="sume")
            nc.vector.tensor_reduce(
                out=sume[:], in_=exl[:], axis=mybir.AxisListType.X,
                op=mybir.AluOpType.add,
            )
            lns = sp.tile([P, TPC], f32, tag="lns")
            nc.scalar.activation(out=lns[:], in_=sume[:], func=Act.Ln)
            nc.vector.tensor_tensor(
                out=lg_all[:], in0=lg_all[:],
                in1=lns[:, :, None].to_broadcast([P, TPC, NCLASS]),
                op=mybir.AluOpType.subtract,
            )
            nc.sync.dma_start(
                out=out.rearrange("(t p) c -> p t c", p=P), in_=lg_all[:]
            )

    nc.compile()
    return nc


def _prepare(inputs):
    x = np.asarray(inputs["x"], np.float32)
    Wg = np.asarray(inputs["Wg"], np.float32)
    att_src = np.asarray(inputs["att_src"], np.float32)
    att_dst = np.asarray(inputs["att_dst"], np.float32)
    bg = np.asarray(inputs["bg"], np.float32)
    Wc = np.asarray(inputs["Wc"], np.float32)
    bc = np.asarray(inputs["bc"], np.float32)
    Wl = np.asarray(inputs["Wl"], np.float32)
    bl = np.asarray(inputs["bl"], np.float32)
    edge_index = np.asarray(inputs["edge_index"])

    st = _build_structures(edge_index)

    # fold attention vectors into the feature matmul: a_s = x @ (Wg @ As)
    As = np.zeros((HC, H), np.float32)
    Ad = np.zeros((HC, H), np.float32)
    for h in range(H):
        As[h * C:(h + 1) * C, h] = att_src[h]
        Ad[h * C:(h + 1) * C, h] = att_dst[h]
    wg_aug = np.concatenate(
        [Wg, Wg @ As, np.zeros((F_IN, TABW - HC - H), np.float32)], axis=1
    )  # [128, 384], zero-padded so phase A initializes full table rows
    wg_ad = Wg @ Ad                                 # [128, 4]

    x_padT = np.zeros((F_IN, NPAD), np.float32)
    x_padT[:, :N] = x.T

    bf = ml_dtypes.bfloat16
    in_maps = []
    for c in range(NCORES):
        xpT = np.zeros((F_IN, S), np.float32)
        xpT[:, :NPC] = x[st["perm"][c]].T
        dv = np.zeros((P, TPC), np.float32)
        dvp = np.zeros(S, np.float32)
        dvp[:NPC] = st["dinv"][st["perm"][c]]
        dv[:, :] = dvp.reshape(TPC, P).T
        dv2 = np.zeros((P, TPC), np.float32)
        dvp2 = np.zeros(S, np.float32)
        dvp2[:NPC] = st["dinv"][st["perm2"][c]]
        dv2[:, :] = dvp2.reshape(TPC, P).T
        in_maps.append({
            "x_padT": x_padT.astype(bf),
            "x_permT": xpT.astype(bf),
            "dinv_pt": dv,
            "dinv2_pt": dv2,
            "gat_idx": st["gat_idx"][c],
            "gcn_idx": st["gcn_idx"][c],
            "wg_aug": wg_aug.astype(bf),
            "wg_ad": wg_ad.astype(bf),
            "wc": Wc.astype(bf),
            "wl": Wl.astype(bf),
            "bg_b": np.tile(bg[None, :], (P, 1)),
            "bc_b": np.tile(bc[None, :], (P, 1)),
            "bl_b": np.tile(bl[None, :], (P, 1)),
            "ident_bf": np.eye(P, dtype=bf),
            "ident_f": np.eye(P, dtype=np.float32),
        })
    return st, in_maps


def _run(inputs, trace=False, trace_kwargs=None):
    st, in_maps = _prepare(inputs)
    nc = _build_kernel(
        st["Dlo"], st["Dhi"], st["D2lo"], st["D2hi"],
        st["gat_idx"][0].shape[1], st["gcn_idx"][0].shape[1],
    )
    res = run_bass_kernel_spmd(
        nc, in_maps, list(range(NCORES)), trace=trace, **(trace_kwargs or {})
    )
    out = np.empty((N, NCLASS), np.float32)
    for c in range(NCORES):
        out[st["perm2"][c]] = res.results[c]["out"][:NPC]
    return out, res


def kernel(**inputs) -> np.ndarray:
    out, _ = _run(inputs, trace=False)
    return out


# revision 17
# speedup vs baseline: 1.0775x; 1.0072x over previous
"""GAT + GCN + classifier over a COO graph, distributed over 8 TRN2 NeuronCores.

v2 strategy (descriptor- and instruction-count driven):
  - Nodes dealt to 8 cores by (d_lo1, d_hi1) lexsort round-robin (balances
    both the per-core edge count and the per-tile degree profiles).
  - Phase A: every core builds the full gather table T[n] = [h(n) | a_s(n)]
    (bf16, 768B rows) from x^T tiles (host-pretransposed, so no on-device
    transposes); PSUM->SBUF casts alternate DVE/ACT, paired 2 tiles/instr.
  - GAT phase: per dst-tile slot-major dma_gather (lo/hi int16 halves),
    softmax fused into ~7 wide instructions (ACT Lrelu + Exp, DVE broadcast
    adds + reduces), weighted sum via broadcast multiply + pairwise tree.
    ELU + bias deferred to one bulk pass per 8-tile group.
  - GCN phase: u = dinv*(xg @ Wc) AllGathered, then TRANSPOSED dma_gather
    (lane-major columns) -> per-lane tensor_reduce gives xc^T directly ->
    ACT relu(+bc) -> classifier matmul without any transposes. dst nodes
    re-sorted per core by GCN degree profile (perm2) to cut slot padding.
  - log_softmax bulk at the end; host de-permutes rows (perm2 order).
"""
import sys

sys.path.insert(0, "/opt/trn_rl_repo")

import numpy as np
import ml_dtypes

import concourse.bass as bass
import concourse.bacc as bacc
import concourse.mybir as mybir
import concourse.tile as tile
from concourse.bass_utils import run_bass_kernel_spmd

# problem constants (hardcoded per contract)
N = 50000
E = 800000
F_IN = 128
H = 4
C = 64
HC = H * C          # 256
HID = 128
NCLASS = 10
NEG = 0.2

NCORES = 8
P = 128
NPC = N // NCORES   # 6250 nodes per core
TPC = 49            # tiles per core (49*128 = 6272 >= 6250)
S = TPC * P         # 6272 padded slots per core
SPLIT = 32768       # int16 gather index range per table half
XT = 391            # x tiles for table build (391*128 = 50048)
NPAD = XT * P       # 50048
NT_ROWS = 1 + NPAD + 1   # gather table rows: [dummy | nodes (+pad) | hi dummy]
HI_DUMMY = NT_ROWS - 1   # 50049
TABW = 384          # bf16 table row: 0:256 h, 256:260 a_s, 260:384 junk pad
ASD_NEG = -10000.0  # a_s marker for dummy rows (drives softmax weight to ~0)
NU_ROWS = NCORES * S     # 50176 u-table rows
U_LO_DUMMY = NPC         # row 6250 (core0 pad slot -> always zero)
U_HI_DUMMY = 7 * S + NPC # row 50154 (core7 pad slot)
GRP = 8             # GAT tiles per ELU/phase-C group

f32 = mybir.dt.float32
bf16 = mybir.dt.bfloat16
i16 = mybir.dt.int16


def _build_structures(edge_index):
    src = np.asarray(edge_index[0], dtype=np.int64)
    dst = np.asarray(edge_index[1], dtype=np.int64)
    src = np.concatenate([src, np.arange(N, dtype=np.int64)])
    dst = np.concatenate([dst, np.arange(N, dtype=np.int64)])
    deg = np.bincount(dst, minlength=N).astype(np.int64)
    dinv = (1.0 / np.sqrt(deg)).astype(np.float32)

    indptr = np.zeros(N + 1, np.int64)
    np.cumsum(deg, out=indptr[1:])

    # --- GAT split: table row of node n follows the interleaved phase-A
    #     write pattern (chunk i4 of 4 x-tiles: row = 1 + i4*512 + p*n_t + k)
    row_of = np.empty(NPAD, np.int64)
    for i4 in range((XT + 3) // 4):
        n_t = min(4, XT - i4 * 4)
        for k in range(n_t):
            cols = (i4 * 4 + k) * P + np.arange(P)
            row_of[cols] = 1 + i4 * 4 * P + np.arange(P) * n_t + k
    rr = row_of[src]
    hi1 = rr >= SPLIT
    d_hi1 = np.bincount(dst[hi1], minlength=N).astype(np.int64)
    d_lo1 = deg - d_hi1
    order1 = np.lexsort((hi1, dst))
    adj1 = rr[order1]  # table rows, grouped by dst, lo sources first

    # --- node -> core deal by (d_lo1, d_hi1) lexsort (matches tile profiles
    #     across cores), then per-core sort by the same keys ---
    key_order = np.lexsort((d_hi1, d_lo1))
    perm = np.empty((NCORES, NPC), np.int64)
    for c in range(NCORES):
        nodes = key_order[c::NCORES]
        k = np.lexsort((d_hi1[nodes], d_lo1[nodes]))
        perm[c] = nodes[k]
    pos = np.empty(N, np.int64)
    for c in range(NCORES):
        pos[perm[c]] = c * S + np.arange(NPC)

    # --- GCN split: u-table row of node n is pos[n] ---
    ps = pos[src]
    hi2 = ps >= SPLIT
    d_hi2 = np.bincount(dst[hi2], minlength=N).astype(np.int64)
    d_lo2 = deg - d_hi2
    order2 = np.lexsort((hi2, dst))
    adj2 = ps[order2]  # u-table positions, grouped by dst, lo first

    # --- per-core GCN re-sort (perm2): same node set, ordered by GCN keys ---
    perm2 = np.empty((NCORES, NPC), np.int64)
    for c in range(NCORES):
        nodes = perm[c]
        k = np.lexsort((d_hi2[nodes], d_lo2[nodes]))
        perm2[c] = nodes[k]

    # --- common (max across cores) per-tile slot profiles ---
    def tile_prof(dvals, pm):
        m = np.zeros((NCORES, S), np.int64)
        for c in range(NCORES):
            m[c, :NPC] = dvals[pm[c]]
        return m.reshape(NCORES, TPC, P).max(axis=(0, 2))

    Dlo = tile_prof(d_lo1, perm)
    Dhi = tile_prof(d_hi1, perm)
    D2lo = tile_prof(d_lo2, perm2)
    D2hi = tile_prof(d_hi2, perm2)

    def block(nodes, Dt, dcount, base, adj, shift, dummy, lane_major):
        """Padded [Dt*128] int index block for one tile."""
        if Dt == 0:
            return np.zeros(0, np.int64)
        nv = np.maximum(nodes, 0)
        cnt = np.where(nodes >= 0, dcount[nv], 0)
        sl = np.arange(Dt)
        ei = base[:, None] + sl[None, :]
        valid = sl[None, :] < cnt[:, None]
        vals = np.where(valid, adj[np.where(valid, ei, 0)] + shift, dummy)
        if lane_major:
            return vals.reshape(-1)       # position = lane*Dt + slot
        return vals.T.reshape(-1)         # position = slot*128 + lane

    def wrap16(flat):
        # position i -> [i % 16, i // 16], replicated to 128 partitions
        arr = flat.reshape(-1, 16).T
        return np.tile(arr, (8, 1))

    gat_idx = []
    gcn_idx = []
    for c in range(NCORES):
        nodes_pad = np.full(S, -1, np.int64)
        nodes_pad[:NPC] = perm[c]
        nodes_pad2 = np.full(S, -1, np.int64)
        nodes_pad2[:NPC] = perm2[c]
        cols1 = []
        cols2 = []
        for t in range(TPC):
            nodes = nodes_pad[t * P:(t + 1) * P]
            nv = np.maximum(nodes, 0)
            b_lo1 = indptr[nv]
            b_hi1 = indptr[nv] + d_lo1[nv]
            lo = block(nodes, Dlo[t], d_lo1, b_lo1, adj1, 0, 0, False)
            hi = block(nodes, Dhi[t], d_hi1, b_hi1, adj1, -SPLIT,
                       HI_DUMMY - SPLIT, False)
            assert lo.size == 0 or (0 <= lo.min() and lo.max() < SPLIT)
            assert hi.size == 0 or (0 <= hi.min() and hi.max() <= HI_DUMMY - SPLIT)
            cols1.append(wrap16(lo))
            cols1.append(wrap16(hi))

            nodes2 = nodes_pad2[t * P:(t + 1) * P]
            nv2 = np.maximum(nodes2, 0)
            b_lo2 = indptr[nv2]
            b_hi2 = indptr[nv2] + d_lo2[nv2]
            lo2 = block(nodes2, D2lo[t], d_lo2, b_lo2, adj2, 0, U_LO_DUMMY, False)
            hi2b = block(nodes2, D2hi[t], d_hi2, b_hi2, adj2, -SPLIT,
                         U_HI_DUMMY - SPLIT, False)
            assert lo2.size == 0 or (0 <= lo2.min() and lo2.max() < SPLIT)
            assert hi2b.size == 0 or (0 <= hi2b.min() and hi2b.max() < SPLIT)
            cols2.append(wrap16(lo2))
            cols2.append(wrap16(hi2b))
        gat_idx.append(np.concatenate(cols1, axis=1).astype(np.int16))
        gcn_idx.append(np.concatenate(cols2, axis=1).astype(np.int16))

    return dict(
        dinv=dinv, perm=perm, perm2=perm2,
        Dlo=Dlo.tolist(), Dhi=Dhi.tolist(),
        D2lo=D2lo.tolist(), D2hi=D2hi.tolist(),
        gat_idx=gat_idx, gcn_idx=gcn_idx,
    )


def _build_kernel(Dlo, Dhi, D2lo, D2hi, gat_cols, gcn_cols):
    nc = bacc.Bacc(None, num_devices=NCORES, num_swdge_queues=4)

    x_padT = nc.declare_dram_parameter("x_padT", [F_IN, NPAD], bf16, isOutput=False)
    x_permT = nc.declare_dram_parameter("x_permT", [F_IN, S], bf16, isOutput=False)
    dinv_pt = nc.declare_dram_parameter("dinv_pt", [P, TPC], f32, isOutput=False)
    dinv2_pt = nc.declare_dram_parameter("dinv2_pt", [P, TPC], f32, isOutput=False)
    gat_idx = nc.declare_dram_parameter("gat_idx", [P, gat_cols], i16, isOutput=False)
    gcn_idx = nc.declare_dram_parameter("gcn_idx", [P, gcn_cols], i16, isOutput=False)
    wg_aug = nc.declare_dram_parameter("wg_aug", [F_IN, TABW], bf16, isOutput=False)
    wg_ad = nc.declare_dram_parameter("wg_ad", [F_IN, H], bf16, isOutput=False)
    wc = nc.declare_dram_parameter("wc", [HC, HID], bf16, isOutput=False)
    wl = nc.declare_dram_parameter("wl", [HID, NCLASS], bf16, isOutput=False)
    bg_b = nc.declare_dram_parameter("bg_b", [P, HC], f32, isOutput=False)
    bc_b = nc.declare_dram_parameter("bc_b", [P, HID], f32, isOutput=False)
    bl_b = nc.declare_dram_parameter("bl_b", [P, NCLASS], f32, isOutput=False)
    ident_bf_in = nc.declare_dram_parameter("ident_bf", [P, P], bf16, isOutput=False)
    ident_f_in = nc.declare_dram_parameter("ident_f", [P, P], f32, isOutput=False)
    out = nc.declare_dram_parameter("out", [S, NCLASS], f32, isOutput=True)

    h_table = nc.dram_tensor("h_table", [NT_ROWS, TABW], bf16)
    ag_in = nc.dram_tensor("ag_in", [S, HID], bf16)
    ag_out = nc.dram_tensor("ag_out", [NU_ROWS, HID], bf16, addr_space="Shared")

    AW = TABW
    Act = mybir.ActivationFunctionType

    with tile.TileContext(nc) as tc:
        with (
            tc.tile_pool(name="const", bufs=1) as cpool,
            tc.tile_pool(name="sbuf", bufs=3) as sb,
            tc.tile_pool(name="gat", bufs=2) as gp,
            tc.tile_pool(name="scratch", bufs=1) as sp,
            tc.tile_pool(name="softmax", bufs=3) as spE,
            tc.tile_pool(name="gut", bufs=4) as gu,
            tc.tile_pool(name="psA", bufs=2, space="PSUM") as ppA,
            tc.tile_pool(name="psB", bufs=2, space="PSUM") as ppB,
        ):
            # ---- resident constants ----
            ident_bf = cpool.tile([P, P], bf16)
            nc.sync.dma_start(out=ident_bf[:], in_=ident_bf_in[:])
            wga_t = cpool.tile([F_IN, AW], bf16)
            nc.sync.dma_start(out=wga_t[:], in_=wg_aug[:])
            wgad_t = cpool.tile([F_IN, H], bf16)
            nc.sync.dma_start(out=wgad_t[:], in_=wg_ad[:])
            wc_t = cpool.tile([P, 2, HID], bf16)
            nc.sync.dma_start(out=wc_t[:], in_=wc.rearrange("(k p) n -> p k n", p=P))
            wlb_t = cpool.tile([HID, NCLASS], bf16)
            nc.sync.dma_start(out=wlb_t[:], in_=wl[:])
            bg_t = cpool.tile([P, HC], f32)
            nc.sync.dma_start(out=bg_t[:], in_=bg_b[:])
            bc_t = cpool.tile([P, HID], f32)
            nc.sync.dma_start(out=bc_t[:], in_=bc_b[:])
            ident_f = cpool.tile([P, P], f32)
            nc.sync.dma_start(out=ident_f[:], in_=ident_f_in[:])
            bl_t = cpool.tile([P, NCLASS], f32)
            nc.sync.dma_start(out=bl_t[:], in_=bl_b[:])
            dinv_t = cpool.tile([P, TPC], f32)
            nc.sync.dma_start(out=dinv_t[:], in_=dinv_pt[:])
            dinv2_t = cpool.tile([P, TPC], f32)
            nc.sync.dma_start(out=dinv2_t[:], in_=dinv2_pt[:])
            ad_all = cpool.tile([P, TPC, H], f32)
            xg_all = cpool.tile([P, TPC, HC], bf16)
            lg_all = cpool.tile([P, TPC, NCLASS], f32)

            # ---- dummy table rows ----
            dum = cpool.tile([1, TABW], bf16)
            nc.vector.memset(dum[:], 0.0)
            nc.vector.memset(dum[:, HC:HC + H], ASD_NEG)
            nc.sync.dma_start(out=h_table[0:1, :], in_=dum[:])
            nc.sync.dma_start(out=h_table[HI_DUMMY:HI_DUMMY + 1, :], in_=dum[:])

            # ---- phase A: build gather table rows 1..NPAD ----
            # 4 tiles per chunk: one 1KB-contiguous x read and one 3KB-
            # contiguous interleaved table write (row of node = host pi map).
            for i4 in range((XT + 3) // 4):
                n_t = min(4, XT - i4 * 4)
                xq = sb.tile([P, 4 * P], bf16, tag="xq")
                nc.sync.dma_start(
                    out=xq[:, 0:n_t * P],
                    in_=x_padT[:, i4 * 4 * P:(i4 * 4 + n_t) * P],
                )
                hbf = sb.tile([P, 4, AW], bf16, tag="hbf")
                for k2 in range((n_t + 1) // 2):
                    n_p = min(2, n_t - k2 * 2)
                    hps = ppA.tile([P, 2, 512], f32, tag="mm_psA")
                    for k in range(n_p):
                        nc.tensor.matmul(
                            hps[:, k, 0:AW],
                            lhsT=xq[:, (k2 * 2 + k) * P:(k2 * 2 + k + 1) * P],
                            rhs=wga_t[:], start=True, stop=True,
                        )
                    src_ap = hps[:, 0:n_p, 0:AW]
                    dst_ap = hbf[:, k2 * 2:k2 * 2 + n_p, :]
                    if k2 % 2 == 0:
                        nc.vector.tensor_copy(out=dst_ap, in_=src_ap)
                    else:
                        nc.scalar.activation(out=dst_ap, in_=src_ap, func=Act.Copy)
                nc.sync.dma_start(
                    out=h_table[
                        1 + i4 * 4 * P:1 + (i4 * 4 + n_t) * P, :
                    ].rearrange("(p k) w -> p k w", k=n_t),
                    in_=hbf[:, 0:n_t, :],
                )

            # ---- phase A2: per-tile a_d for this core's own nodes ----
            for t in range(TPC):
                xT = sb.tile([P, P], bf16, tag="xT")
                nc.sync.dma_start(out=xT[:], in_=x_permT[:, t * P:(t + 1) * P])
                adps = ppB.tile([P, H], f32, tag="mm_psB")
                nc.tensor.matmul(adps[:], lhsT=xT[:], rhs=wgad_t[:],
                                 start=True, stop=True)
                nc.vector.tensor_copy(out=ad_all[:, t, :], in_=adps[:])

            # ---- phase B: GAT per tile; ELU + phase C per group of 8 ----
            qrr = [0]  # swdge queue round-robin
            goff = 0
            for g0 in range(0, TPC, GRP):
                g1 = min(g0 + GRP, TPC)
                for t in range(g0, g1):
                    dlo, dhi = Dlo[t], Dhi[t]
                    D = dlo + dhi
                    w = 8 * D
                    idx_t = gp.tile([P, w], i16, tag="gidx")
                    nc.sync.dma_start(out=idx_t[:], in_=gat_idx[:, goff:goff + w])
                    goff += w
                    G = gp.tile([P, D, TABW], bf16, tag="G")
                    if dlo > 0:
                        nc.gpsimd.dma_gather(
                            out_ap=G[:, 0:dlo, :],
                            in_ap=h_table[:, :],
                            idxs_ap=idx_t[:, 0:8 * dlo],
                            num_idxs=P * dlo,
                            num_idxs_reg=P * dlo,
                            elem_size=TABW,
                            single_packet=False,
                            queue_num=qrr[0] % 4,
                        )
                        qrr[0] += 1
                    if dhi > 0:
                        nc.gpsimd.dma_gather(
                            out_ap=G[:, dlo:D, :],
                            in_ap=h_table[SPLIT:, :],
                            idxs_ap=idx_t[:, 8 * dlo:w],
                            num_idxs=P * dhi,
                            num_idxs_reg=P * dhi,
                            elem_size=TABW,
                            single_packet=False,
                            queue_num=qrr[0] % 4,
                        )
                        qrr[0] += 1
                    # e = leaky_relu(a_s[src] + a_d[dst])   [P, H, D]
                    e = spE.tile([P, H, D], f32, tag="e")
                    nc.vector.tensor_tensor(
                        out=e[:],
                        in0=G[:, :, HC:HC + H].rearrange("p d h -> p h d"),
                        in1=ad_all[:, t, :][:, :, None].to_broadcast([P, H, D]),
                        op=mybir.AluOpType.add,
                    )
                    e2 = spE.tile([P, H, D], f32, tag="e2")
                    nc.vector.tensor_scalar(
                        out=e2[:], in0=e[:], scalar1=NEG, scalar2=None,
                        op0=mybir.AluOpType.mult,
                    )
                    nc.vector.tensor_tensor(
                        out=e[:], in0=e[:], in1=e2[:], op=mybir.AluOpType.max
                    )
                    negm = spE.tile([P, H], f32, tag="negm")
                    nc.vector.tensor_reduce(
                        out=negm[:], in_=e[:], axis=mybir.AxisListType.X,
                        op=mybir.AluOpType.max, negate=True,
                    )
                    ex = spE.tile([P, H, D], f32, tag="ex")
                    nc.vector.tensor_tensor(
                        out=ex[:], in0=e[:],
                        in1=negm[:, :, None].to_broadcast([P, H, D]),
                        op=mybir.AluOpType.add,
                    )
                    nc.scalar.activation(out=ex[:], in_=ex[:], func=Act.Exp)
                    den = spE.tile([P, H], f32, tag="den")
                    nc.vector.tensor_reduce(
                        out=den[:], in_=ex[:], axis=mybir.AxisListType.X,
                        op=mybir.AluOpType.add,
                    )
                    rden = spE.tile([P, H], f32, tag="rden")
                    nc.vector.reciprocal(rden[:], den[:])
                    # prod[p, d, h, c] = h_gathered * ex  (bf16)
                    prod = sp.tile([P, D, HC], bf16, tag="prod")
                    g_h = G[:, :, 0:HC].rearrange("p d (h c) -> p d h c", h=H)
                    ex_b = ex.rearrange("p h d -> p d h")[:, :, :, None].to_broadcast(
                        [P, D, H, C]
                    )
                    nc.vector.tensor_tensor(
                        out=prod.rearrange("p d (h c) -> p d h c", h=H),
                        in0=g_h, in1=ex_b, op=mybir.AluOpType.mult,
                    )
                    # tree-reduce over D slots, in place on prod (bf16
                    # until width <= 6, then one f32 level)
                    cur = D
                    while cur > 6:
                        h2 = cur // 2
                        nc.vector.tensor_tensor(
                            out=prod[:, 0:h2, :], in0=prod[:, 0:h2, :],
                            in1=prod[:, h2:2 * h2, :], op=mybir.AluOpType.add,
                        )
                        if cur % 2:
                            nc.vector.tensor_tensor(
                                out=prod[:, 0, :], in0=prod[:, 0, :],
                                in1=prod[:, 2 * h2, :], op=mybir.AluOpType.add,
                            )
                        cur = h2
                    acc = sp.tile([P, HC], f32, tag="accF")
                    if cur == 1:
                        nc.vector.tensor_copy(out=acc[:], in_=prod[:, 0, :])
                    else:
                        nc.vector.tensor_tensor(
                            out=acc[:], in0=prod[:, 0, :], in1=prod[:, 1, :],
                            op=mybir.AluOpType.add,
                        )
                        for j in range(2, cur):
                            nc.vector.tensor_tensor(
                                out=acc[:], in0=acc[:], in1=prod[:, j, :],
                                op=mybir.AluOpType.add,
                            )
                    # xg_raw = acc / den  -> bulk buffer (bf16)
                    nc.vector.tensor_tensor(
                        out=xg_all[:, t, :].rearrange("p (h c) -> p h c", h=H),
                        in0=acc.rearrange("p (h c) -> p h c", h=H),
                        in1=rden[:, :, None].to_broadcast([P, H, C]),
                        op=mybir.AluOpType.mult,
                    )

                # ---- group ELU: xg = elu(xg_raw + bg) in-place (bf16) ----
                ng = g1 - g0
                xs = xg_all[:, g0:g1, :]
                nc.vector.tensor_tensor(
                    out=xs, in0=xs,
                    in1=bg_t[:, None, :].to_broadcast([P, ng, HC]),
                    op=mybir.AluOpType.add,
                )
                tneg = sp.tile([P, GRP * HC], f32, tag="tneg")
                tn = tneg[:, 0:ng * HC].rearrange("p (g c) -> p g c", g=ng)
                nc.vector.tensor_scalar(
                    out=tn, in0=xs, scalar1=0.0, scalar2=None,
                    op0=mybir.AluOpType.min,
                )
                nc.scalar.activation(out=tn, in_=tn, func=Act.Exp)
                nc.vector.tensor_scalar(
                    out=xs, in0=xs, scalar1=0.0, scalar2=None,
                    op0=mybir.AluOpType.max,
                )
                nc.vector.tensor_tensor(
                    out=xs, in0=xs, in1=tn, op=mybir.AluOpType.add,
                )
                nc.vector.tensor_scalar(
                    out=xs, in0=xs, scalar1=1.0, scalar2=None,
                    op0=mybir.AluOpType.subtract,
                )

                # ---- phase C for the group: u = dinv * (xg @ Wc) ----
                for t in range(g0, g1):
                    xwps = ppB.tile([P, HID], f32, tag="mm_psB")
                    for k in range(2):
                        xgT_ps = ppA.tile([P, P], bf16, tag="tr_ps")
                        nc.tensor.transpose(
                            xgT_ps[:], xg_all[:, t, k * P:(k + 1) * P], ident_bf[:]
                        )
                        xgT = sb.tile([P, P], bf16, tag="xgT")
                        nc.scalar.activation(
                            out=xgT[:], in_=xgT_ps[:], func=Act.Copy
                        )
                        nc.tensor.matmul(
                            xwps[:], lhsT=xgT[:], rhs=wc_t[:, k, :],
                            start=(k == 0), stop=(k == 1),
                        )
                    ub = gp.tile([P, HID], bf16, tag="ub")
                    nc.vector.tensor_scalar(
                        out=ub[:], in0=xwps[:], scalar1=dinv_t[:, t:t + 1],
                        scalar2=None, op0=mybir.AluOpType.mult,
                    )
                    nc.sync.dma_start(out=ag_in[t * P:(t + 1) * P, :], in_=ub[:])

            # ---- phase D: AllGather u across cores ----
            nc.gpsimd.collective_compute(
                "AllGather",
                mybir.AluOpType.bypass,
                replica_groups=[list(range(NCORES))],
                ins=[ag_in[:]],
                outs=[ag_out[:]],
            )

            # ---- phase E: GCN via slot-major gathers + classifier ----
            goff = 0
            for t in range(TPC):
                dlo, dhi = D2lo[t], D2hi[t]
                D = dlo + dhi
                w = 8 * D
                idx_t = gu.tile([P, w], i16, tag="gidx2")
                nc.sync.dma_start(out=idx_t[:], in_=gcn_idx[:, goff:goff + w])
                goff += w
                Gu = gu.tile([P, D, HID], bf16, tag="Gu")
                if dlo > 0:
                    nc.gpsimd.dma_gather(
                        out_ap=Gu[:, 0:dlo, :],
                        in_ap=ag_out[:, :],
                        idxs_ap=idx_t[:, 0:8 * dlo],
                        num_idxs=P * dlo,
                        num_idxs_reg=P * dlo,
                        elem_size=HID,
                        single_packet=False,
                        queue_num=qrr[0] % 4,
                    )
                    qrr[0] += 1
                if dhi > 0:
                    nc.gpsimd.dma_gather(
                        out_ap=Gu[:, dlo:D, :],
                        in_ap=ag_out[SPLIT:, :],
                        idxs_ap=idx_t[:, 8 * dlo:w],
                        num_idxs=P * dhi,
                        num_idxs_reg=P * dhi,
                        elem_size=HID,
                        single_packet=False,
                        queue_num=qrr[0] % 4,
                    )
                    qrr[0] += 1
                # tree-reduce over D slots in place (bf16), tail in f32
                cur = D
                while cur > 6:
                    h2 = cur // 2
                    nc.vector.tensor_tensor(
                        out=Gu[:, 0:h2, :], in0=Gu[:, 0:h2, :],
                        in1=Gu[:, h2:2 * h2, :], op=mybir.AluOpType.add,
                    )
                    if cur % 2:
                        nc.vector.tensor_tensor(
                            out=Gu[:, 0, :], in0=Gu[:, 0, :],
                            in1=Gu[:, 2 * h2, :], op=mybir.AluOpType.add,
                        )
                    cur = h2
                uacc = sp.tile([P, HID], f32, tag="uaccF")
                if cur == 1:
                    nc.vector.tensor_copy(out=uacc[:], in_=Gu[:, 0, :])
                else:
                    nc.vector.tensor_tensor(
                        out=uacc[:], in0=Gu[:, 0, :], in1=Gu[:, 1, :],
                        op=mybir.AluOpType.add,
                    )
                    for j in range(2, cur):
                        nc.vector.tensor_tensor(
                            out=uacc[:], in0=uacc[:], in1=Gu[:, j, :],
                            op=mybir.AluOpType.add,
                        )
                # xc = relu(dinv * sum + bc), bf16 for the transpose
                xc = gu.tile([P, HID], bf16, tag="xc")
                nc.vector.tensor_scalar(
                    out=uacc[:], in0=uacc[:], scalar1=dinv2_t[:, t:t + 1],
                    scalar2=None, op0=mybir.AluOpType.mult,
                )
                nc.vector.tensor_tensor(
                    out=uacc[:], in0=uacc[:], in1=bc_t[:], op=mybir.AluOpType.add
                )
                nc.vector.tensor_scalar(
                    out=xc[:], in0=uacc[:], scalar1=0.0, scalar2=None,
                    op0=mybir.AluOpType.max,
                )
                # classifier: transpose xc, then bf16 matmul
                xcT_ps = ppA.tile([P, P], bf16, tag="tr_ps")
                nc.tensor.transpose(xcT_ps[:], xc[:], ident_bf[:])
                xcT = sb.tile([P, P], bf16, tag="xcT")
                nc.scalar.activation(out=xcT[:], in_=xcT_ps[:], func=Act.Copy)
                lps = ppB.tile([P, NCLASS], f32, tag="mm_psB")
                nc.tensor.matmul(lps[:], lhsT=xcT[:], rhs=wlb_t[:],
                                 start=True, stop=True)
                nc.vector.tensor_tensor(
                    out=lg_all[:, t, :], in0=lps[:], in1=bl_t[:],
                    op=mybir.AluOpType.add,
                )

            # ---- bulk log_softmax over all tiles ----
            nmx = sp.tile([P, TPC], f32, tag="nmx")
            nc.vector.tensor_reduce(
                out=nmx[:], in_=lg_all[:], axis=mybir.AxisListType.X,
                op=mybir.AluOpType.max, negate=True,
            )
            nc.vector.tensor_tensor(
                out=lg_all[:], in0=lg_all[:],
                in1=nmx[:, :, None].to_broadcast([P, TPC, NCLASS]),
                op=mybir.AluOpType.add,
            )
            exl = sp.tile([P, TPC, NCLASS], f32, tag="exl")
            nc.scalar.activation(out=exl[:], in_=lg_all[:], func=Act.Exp)
            sume = sp.tile([P, TPC], f32, tag="sume")
            nc.vector.tensor_reduce(
                out=sume[:], in_=exl[:], axis=mybir.AxisListType.X,
                op=mybir.AluOpType.add,
            )
            lns = sp.tile([P, TPC], f32, tag="lns")
            nc.scalar.activation(out=lns[:], in_=sume[:], func=Act.Ln)
            nc.vector.tensor_tensor(
                out=lg_all[:], in0=lg_all[:],
                in1=lns[:, :, None].to_broadcast([P, TPC, NCLASS]),
                op=mybir.AluOpType.subtract,
            )
            nc.sync.dma_start(
                out=out.rearrange("(t p) c -> p t c", p=P), in_=lg_all[:]
            )

    nc.compile()
    return nc


def _prepare(inputs):
    x = np.asarray(inputs["x"], np.float32)
    Wg = np.asarray(inputs["Wg"], np.float32)
    att_src = np.asarray(inputs["att_src"], np.float32)
    att_dst = np.asarray(inputs["att_dst"], np.float32)
    bg = np.asarray(inputs["bg"], np.float32)
    Wc = np.asarray(inputs["Wc"], np.float32)
    bc = np.asarray(inputs["bc"], np.float32)
    Wl = np.asarray(inputs["Wl"], np.float32)
    bl = np.asarray(inputs["bl"], np.float32)
    edge_index = np.asarray(inputs["edge_index"])

    st = _build_structures(edge_index)

    # fold attention vectors into the feature matmul: a_s = x @ (Wg @ As)
    As = np.zeros((HC, H), np.float32)
    Ad = np.zeros((HC, H), np.float32)
    for h in range(H):
        As[h * C:(h + 1) * C, h] = att_src[h]
        Ad[h * C:(h + 1) * C, h] = att_dst[h]
    wg_aug = np.concatenate(
        [Wg, Wg @ As, np.zeros((F_IN, TABW - HC - H), np.float32)], axis=1
    )  # [128, 384], zero-padded so phase A initializes full table rows
    wg_ad = Wg @ Ad                                 # [128, 4]

    x_padT = np.zeros((F_IN, NPAD), np.float32)
    x_padT[:, :N] = x.T

    bf = ml_dtypes.bfloat16
    in_maps = []
    for c in range(NCORES):
        xpT = np.zeros((F_IN, S), np.float32)
        xpT[:, :NPC] = x[st["perm"][c]].T
        dv = np.zeros((P, TPC), np.float32)
        dvp = np.zeros(S, np.float32)
        dvp[:NPC] = st["dinv"][st["perm"][c]]
        dv[:, :] = dvp.reshape(TPC, P).T
        dv2 = np.zeros((P, TPC), np.float32)
        dvp2 = np.zeros(S, np.float32)
        dvp2[:NPC] = st["dinv"][st["perm2"][c]]
        dv2[:, :] = dvp2.reshape(TPC, P).T
        in_maps.append({
            "x_padT": x_padT.astype(bf),
            "x_permT": xpT.astype(bf),
            "dinv_pt": dv,
            "dinv2_pt": dv2,
            "gat_idx": st["gat_idx"][c],
            "gcn_idx": st["gcn_idx"][c],
            "wg_aug": wg_aug.astype(bf),
            "wg_ad": wg_ad.astype(bf),
            "wc": Wc.astype(bf),
            "wl": Wl.astype(bf),
            "bg_b": np.tile(bg[None, :], (P, 1)),
            "bc_b": np.tile(bc[None, :], (P, 1)),
            "bl_b": np.tile(bl[None, :], (P, 1)),
            "ident_bf": np.eye(P, dtype=bf),
            "ident_f": np.eye(P, dtype=np.float32),
        })
    return st, in_maps


def _run(inputs, trace=False, trace_kwargs=None):
    st, in_maps = _prepare(inputs)
    nc = _build_kernel(
        st["Dlo"], st["Dhi"], st["D2lo"], st["D2hi"],
        st["gat_idx"][0].shape[1], st["gcn_idx"][0].shape[1],
    )
    res = run_bass_kernel_spmd(
        nc, in_maps, list(range(NCORES)), trace=trace, **(trace_kwargs or {})
    )
    out = np.empty((N, NCLASS), np.float32)
    for c in range(NCORES):
        out[st["perm2"][c]] = res.results[c]["out"][:NPC]
    return out, res


def kernel(**inputs) -> np.ndarray:
    out, _ = _run(inputs, trace=False)
    return out


# revision 18
# speedup vs baseline: 1.0901x; 1.0117x over previous
"""GAT + GCN + classifier over a COO graph, distributed over 8 TRN2 NeuronCores.

v2 strategy (descriptor- and instruction-count driven):
  - Nodes dealt to 8 cores by (d_lo1, d_hi1) lexsort round-robin (balances
    both the per-core edge count and the per-tile degree profiles).
  - Phase A: every core builds the full gather table T[n] = [h(n) | a_s(n)]
    (bf16, 768B rows) from x^T tiles (host-pretransposed, so no on-device
    transposes); PSUM->SBUF casts alternate DVE/ACT, paired 2 tiles/instr.
  - GAT phase: per dst-tile slot-major dma_gather (lo/hi int16 halves),
    softmax fused into ~7 wide instructions (ACT Lrelu + Exp, DVE broadcast
    adds + reduces), weighted sum via broadcast multiply + pairwise tree.
    ELU + bias deferred to one bulk pass per 8-tile group.
  - GCN phase: u = dinv*(xg @ Wc) AllGathered, then TRANSPOSED dma_gather
    (lane-major columns) -> per-lane tensor_reduce gives xc^T directly ->
    ACT relu(+bc) -> classifier matmul without any transposes. dst nodes
    re-sorted per core by GCN degree profile (perm2) to cut slot padding.
  - log_softmax bulk at the end; host de-permutes rows (perm2 order).
"""
import sys

sys.path.insert(0, "/opt/trn_rl_repo")

import numpy as np
import ml_dtypes

import concourse.bass as bass
import concourse.bacc as bacc
import concourse.mybir as mybir
import concourse.tile as tile
from concourse.bass_utils import run_bass_kernel_spmd

# problem constants (hardcoded per contract)
N = 50000
E = 800000
F_IN = 128
H = 4
C = 64
HC = H * C          # 256
HID = 128
NCLASS = 10
NEG = 0.2

NCORES = 8
P = 128
NPC = N // NCORES   # 6250 nodes per core
TPC = 49            # tiles per core (49*128 = 6272 >= 6250)
S = TPC * P         # 6272 padded slots per core
SPLIT = 32768       # int16 gather index range per table half
XT = 391            # x tiles for table build (391*128 = 50048)
NPAD = XT * P       # 50048
NT_ROWS = 1 + NPAD + 1   # gather table rows: [dummy | nodes (+pad) | hi dummy]
HI_DUMMY = NT_ROWS - 1   # 50049
TABW = 384          # bf16 table row: 0:256 h, 256:260 a_s, 260:384 junk pad
ASD_NEG = -10000.0  # a_s marker for dummy rows (drives softmax weight to ~0)
NU_ROWS = NCORES * S     # 50176 u-table rows
U_LO_DUMMY = NPC         # row 6250 (core0 pad slot -> always zero)
U_HI_DUMMY = 7 * S + NPC # row 50154 (core7 pad slot)
GRP = 8             # GAT tiles per ELU/phase-C group

f32 = mybir.dt.float32
bf16 = mybir.dt.bfloat16
i16 = mybir.dt.int16


def _build_structures(edge_index):
    src = np.asarray(edge_index[0], dtype=np.int64)
    dst = np.asarray(edge_index[1], dtype=np.int64)
    src = np.concatenate([src, np.arange(N, dtype=np.int64)])
    dst = np.concatenate([dst, np.arange(N, dtype=np.int64)])
    deg = np.bincount(dst, minlength=N).astype(np.int64)
    dinv = (1.0 / np.sqrt(deg)).astype(np.float32)

    indptr = np.zeros(N + 1, np.int64)
    np.cumsum(deg, out=indptr[1:])

    # --- GAT split: table row of node n follows the interleaved phase-A
    #     write pattern (chunk i4 of 4 x-tiles: row = 1 + i4*512 + p*n_t + k)
    row_of = np.empty(NPAD, np.int64)
    for i4 in range((XT + 3) // 4):
        n_t = min(4, XT - i4 * 4)
        for k in range(n_t):
            cols = (i4 * 4 + k) * P + np.arange(P)
            row_of[cols] = 1 + i4 * 4 * P + np.arange(P) * n_t + k
    rr = row_of[src]
    hi1 = rr >= SPLIT
    d_hi1 = np.bincount(dst[hi1], minlength=N).astype(np.int64)
    d_lo1 = deg - d_hi1
    order1 = np.lexsort((hi1, dst))
    adj1 = rr[order1]  # table rows, grouped by dst, lo sources first

    # --- node -> core deal by (d_lo1, d_hi1) lexsort (matches tile profiles
    #     across cores), then per-core sort by the same keys ---
    key_order = np.lexsort((d_hi1, d_lo1))
    perm = np.empty((NCORES, NPC), np.int64)
    for c in range(NCORES):
        nodes = key_order[c::NCORES]
        k = np.lexsort((d_hi1[nodes], d_lo1[nodes]))
        perm[c] = nodes[k]
    pos = np.empty(N, np.int64)
    for c in range(NCORES):
        pos[perm[c]] = c * S + np.arange(NPC)

    # --- GCN split: u-table row of node n is pos[n] ---
    ps = pos[src]
    hi2 = ps >= SPLIT
    d_hi2 = np.bincount(dst[hi2], minlength=N).astype(np.int64)
    d_lo2 = deg - d_hi2
    order2 = np.lexsort((hi2, dst))
    adj2 = ps[order2]  # u-table positions, grouped by dst, lo first

    # --- per-core GCN re-sort (perm2): same node set, ordered by GCN keys ---
    perm2 = np.empty((NCORES, NPC), np.int64)
    for c in range(NCORES):
        nodes = perm[c]
        k = np.lexsort((d_hi2[nodes], d_lo2[nodes]))
        perm2[c] = nodes[k]

    # --- common (max across cores) per-tile slot profiles ---
    def tile_prof(dvals, pm):
        m = np.zeros((NCORES, S), np.int64)
        for c in range(NCORES):
            m[c, :NPC] = dvals[pm[c]]
        return m.reshape(NCORES, TPC, P).max(axis=(0, 2))

    Dlo = tile_prof(d_lo1, perm)
    Dhi = tile_prof(d_hi1, perm)
    D2lo = tile_prof(d_lo2, perm2)
    D2hi = tile_prof(d_hi2, perm2)

    def block(nodes, Dt, dcount, base, adj, shift, dummy, lane_major):
        """Padded [Dt*128] int index block for one tile."""
        if Dt == 0:
            return np.zeros(0, np.int64)
        nv = np.maximum(nodes, 0)
        cnt = np.where(nodes >= 0, dcount[nv], 0)
        sl = np.arange(Dt)
        ei = base[:, None] + sl[None, :]
        valid = sl[None, :] < cnt[:, None]
        vals = np.where(valid, adj[np.where(valid, ei, 0)] + shift, dummy)
        if lane_major:
            return vals.reshape(-1)       # position = lane*Dt + slot
        return vals.T.reshape(-1)         # position = slot*128 + lane

    def wrap16(flat):
        # position i -> [i % 16, i // 16], replicated to 128 partitions
        arr = flat.reshape(-1, 16).T
        return np.tile(arr, (8, 1))

    gat_idx = []
    gcn_idx = []
    for c in range(NCORES):
        nodes_pad = np.full(S, -1, np.int64)
        nodes_pad[:NPC] = perm[c]
        nodes_pad2 = np.full(S, -1, np.int64)
        nodes_pad2[:NPC] = perm2[c]
        cols1 = []
        cols2 = []
        for t in range(TPC):
            nodes = nodes_pad[t * P:(t + 1) * P]
            nv = np.maximum(nodes, 0)
            b_lo1 = indptr[nv]
            b_hi1 = indptr[nv] + d_lo1[nv]
            lo = block(nodes, Dlo[t], d_lo1, b_lo1, adj1, 0, 0, False)
            hi = block(nodes, Dhi[t], d_hi1, b_hi1, adj1, -SPLIT,
                       HI_DUMMY - SPLIT, False)
            assert lo.size == 0 or (0 <= lo.min() and lo.max() < SPLIT)
            assert hi.size == 0 or (0 <= hi.min() and hi.max() <= HI_DUMMY - SPLIT)
            cols1.append(wrap16(lo))
            cols1.append(wrap16(hi))

            nodes2 = nodes_pad2[t * P:(t + 1) * P]
            nv2 = np.maximum(nodes2, 0)
            b_lo2 = indptr[nv2]
            b_hi2 = indptr[nv2] + d_lo2[nv2]
            lo2 = block(nodes2, D2lo[t], d_lo2, b_lo2, adj2, 0, U_LO_DUMMY, False)
            hi2b = block(nodes2, D2hi[t], d_hi2, b_hi2, adj2, -SPLIT,
                         U_HI_DUMMY - SPLIT, False)
            assert lo2.size == 0 or (0 <= lo2.min() and lo2.max() < SPLIT)
            assert hi2b.size == 0 or (0 <= hi2b.min() and hi2b.max() < SPLIT)
            cols2.append(wrap16(lo2))
            cols2.append(wrap16(hi2b))
        gat_idx.append(np.concatenate(cols1, axis=1).astype(np.int16))
        gcn_idx.append(np.concatenate(cols2, axis=1).astype(np.int16))

    return dict(
        dinv=dinv, perm=perm, perm2=perm2,
        Dlo=Dlo.tolist(), Dhi=Dhi.tolist(),
        D2lo=D2lo.tolist(), D2hi=D2hi.tolist(),
        gat_idx=gat_idx, gcn_idx=gcn_idx,
    )


def _build_kernel(Dlo, Dhi, D2lo, D2hi, gat_cols, gcn_cols):
    nc = bacc.Bacc(None, num_devices=NCORES, num_swdge_queues=4)

    x_padT = nc.declare_dram_parameter("x_padT", [F_IN, NPAD], bf16, isOutput=False)
    x_permT = nc.declare_dram_parameter("x_permT", [F_IN, S], bf16, isOutput=False)
    dinv_pt = nc.declare_dram_parameter("dinv_pt", [P, TPC], f32, isOutput=False)
    dinv2_pt = nc.declare_dram_parameter("dinv2_pt", [P, TPC], f32, isOutput=False)
    gat_idx = nc.declare_dram_parameter("gat_idx", [P, gat_cols], i16, isOutput=False)
    gcn_idx = nc.declare_dram_parameter("gcn_idx", [P, gcn_cols], i16, isOutput=False)
    wg_aug = nc.declare_dram_parameter("wg_aug", [F_IN, TABW], bf16, isOutput=False)
    wg_ad = nc.declare_dram_parameter("wg_ad", [F_IN, H], bf16, isOutput=False)
    wc = nc.declare_dram_parameter("wc", [HC, HID], bf16, isOutput=False)
    wl = nc.declare_dram_parameter("wl", [HID, NCLASS], bf16, isOutput=False)
    bg_b = nc.declare_dram_parameter("bg_b", [P, HC], f32, isOutput=False)
    bc_b = nc.declare_dram_parameter("bc_b", [P, HID], f32, isOutput=False)
    bl_b = nc.declare_dram_parameter("bl_b", [P, NCLASS], f32, isOutput=False)
    ident_bf_in = nc.declare_dram_parameter("ident_bf", [P, P], bf16, isOutput=False)
    ident_f_in = nc.declare_dram_parameter("ident_f", [P, P], f32, isOutput=False)
    out = nc.declare_dram_parameter("out", [S, NCLASS], f32, isOutput=True)

    h_table = nc.dram_tensor("h_table", [NT_ROWS, TABW], bf16)
    ag_in = nc.dram_tensor("ag_in", [S, HID], bf16)
    ag_out = nc.dram_tensor("ag_out", [NU_ROWS, HID], bf16, addr_space="Shared")

    AW = TABW
    Act = mybir.ActivationFunctionType

    with tile.TileContext(nc) as tc:
        with (
            tc.tile_pool(name="const", bufs=1) as cpool,
            tc.tile_pool(name="sbuf", bufs=3) as sb,
            tc.tile_pool(name="gat", bufs=2) as gp,
            tc.tile_pool(name="scratch", bufs=1) as sp,
            tc.tile_pool(name="softmax", bufs=3) as spE,
            tc.tile_pool(name="gut", bufs=4) as gu,
            tc.tile_pool(name="psA", bufs=2, space="PSUM") as ppA,
            tc.tile_pool(name="psB", bufs=2, space="PSUM") as ppB,
        ):
            # ---- resident constants ----
            ident_bf = cpool.tile([P, P], bf16)
            nc.sync.dma_start(out=ident_bf[:], in_=ident_bf_in[:])
            wga_t = cpool.tile([F_IN, AW], bf16)
            nc.sync.dma_start(out=wga_t[:], in_=wg_aug[:])
            wgad_t = cpool.tile([F_IN, H], bf16)
            nc.sync.dma_start(out=wgad_t[:], in_=wg_ad[:])
            wc_t = cpool.tile([P, 2, HID], bf16)
            nc.sync.dma_start(out=wc_t[:], in_=wc.rearrange("(k p) n -> p k n", p=P))
            wlb_t = cpool.tile([HID, NCLASS], bf16)
            nc.sync.dma_start(out=wlb_t[:], in_=wl[:])
            bg_t = cpool.tile([P, HC], f32)
            nc.sync.dma_start(out=bg_t[:], in_=bg_b[:])
            bc_t = cpool.tile([P, HID], f32)
            nc.sync.dma_start(out=bc_t[:], in_=bc_b[:])
            ident_f = cpool.tile([P, P], f32)
            nc.sync.dma_start(out=ident_f[:], in_=ident_f_in[:])
            bl_t = cpool.tile([P, NCLASS], f32)
            nc.sync.dma_start(out=bl_t[:], in_=bl_b[:])
            dinv_t = cpool.tile([P, TPC], f32)
            nc.sync.dma_start(out=dinv_t[:], in_=dinv_pt[:])
            dinv2_t = cpool.tile([P, TPC], f32)
            nc.sync.dma_start(out=dinv2_t[:], in_=dinv2_pt[:])
            ad_all = cpool.tile([P, TPC, H], f32)
            xg_all = cpool.tile([P, TPC, HC], bf16)
            lg_all = cpool.tile([P, TPC, NCLASS], f32)

            # ---- dummy table rows ----
            dum = cpool.tile([1, TABW], bf16)
            nc.vector.memset(dum[:], 0.0)
            nc.vector.memset(dum[:, HC:HC + H], ASD_NEG)
            nc.sync.dma_start(out=h_table[0:1, :], in_=dum[:])
            nc.sync.dma_start(out=h_table[HI_DUMMY:HI_DUMMY + 1, :], in_=dum[:])

            # ---- phase A: build gather table rows 1..NPAD ----
            # 4 tiles per chunk: one 1KB-contiguous x read and one 3KB-
            # contiguous interleaved table write (row of node = host pi map).
            for i4 in range((XT + 3) // 4):
                n_t = min(4, XT - i4 * 4)
                xq = sb.tile([P, 4 * P], bf16, tag="xq")
                nc.sync.dma_start(
                    out=xq[:, 0:n_t * P],
                    in_=x_padT[:, i4 * 4 * P:(i4 * 4 + n_t) * P],
                )
                hbf = sb.tile([P, 4, AW], bf16, tag="hbf")
                for k2 in range((n_t + 1) // 2):
                    n_p = min(2, n_t - k2 * 2)
                    hps = ppA.tile([P, 2, 512], f32, tag="mm_psA")
                    for k in range(n_p):
                        nc.tensor.matmul(
                            hps[:, k, 0:AW],
                            lhsT=xq[:, (k2 * 2 + k) * P:(k2 * 2 + k + 1) * P],
                            rhs=wga_t[:], start=True, stop=True,
                        )
                    src_ap = hps[:, 0:n_p, 0:AW]
                    dst_ap = hbf[:, k2 * 2:k2 * 2 + n_p, :]
                    if k2 % 2 == 0:
                        nc.vector.tensor_copy(out=dst_ap, in_=src_ap)
                    else:
                        nc.scalar.activation(out=dst_ap, in_=src_ap, func=Act.Copy)
                nc.sync.dma_start(
                    out=h_table[
                        1 + i4 * 4 * P:1 + (i4 * 4 + n_t) * P, :
                    ].rearrange("(p k) w -> p k w", k=n_t),
                    in_=hbf[:, 0:n_t, :],
                )

            # ---- phase A2: per-tile a_d for this core's own nodes ----
            for t in range(TPC):
                xT = sb.tile([P, P], bf16, tag="xT")
                nc.sync.dma_start(out=xT[:], in_=x_permT[:, t * P:(t + 1) * P])
                adps = ppB.tile([P, H], f32, tag="mm_psB")
                nc.tensor.matmul(adps[:], lhsT=xT[:], rhs=wgad_t[:],
                                 start=True, stop=True)
                nc.vector.tensor_copy(out=ad_all[:, t, :], in_=adps[:])

            # ---- phase B: GAT per tile; ELU + phase C per group of 8 ----
            qrr = [0]  # swdge queue round-robin
            goff = 0
            for g0 in range(0, TPC, GRP):
                g1 = min(g0 + GRP, TPC)
                for t in range(g0, g1):
                    dlo, dhi = Dlo[t], Dhi[t]
                    D = dlo + dhi
                    w = 8 * D
                    idx_t = gp.tile([P, w], i16, tag="gidx")
                    nc.sync.dma_start(out=idx_t[:], in_=gat_idx[:, goff:goff + w])
                    goff += w
                    G = gp.tile([P, D, TABW], bf16, tag="G")
                    if dlo > 0:
                        nc.gpsimd.dma_gather(
                            out_ap=G[:, 0:dlo, :],
                            in_ap=h_table[:, :],
                            idxs_ap=idx_t[:, 0:8 * dlo],
                            num_idxs=P * dlo,
                            num_idxs_reg=P * dlo,
                            elem_size=TABW,
                            single_packet=False,
                            queue_num=qrr[0] % 4,
                        )
                        qrr[0] += 1
                    if dhi > 0:
                        nc.gpsimd.dma_gather(
                            out_ap=G[:, dlo:D, :],
                            in_ap=h_table[SPLIT:, :],
                            idxs_ap=idx_t[:, 8 * dlo:w],
                            num_idxs=P * dhi,
                            num_idxs_reg=P * dhi,
                            elem_size=TABW,
                            single_packet=False,
                            queue_num=qrr[0] % 4,
                        )
                        qrr[0] += 1
                    # e = leaky_relu(a_s[src] + a_d[dst])   [P, H, D]
                    e = spE.tile([P, H, D], f32, tag="e")
                    nc.vector.tensor_tensor(
                        out=e[:],
                        in0=G[:, :, HC:HC + H].rearrange("p d h -> p h d"),
                        in1=ad_all[:, t, :][:, :, None].to_broadcast([P, H, D]),
                        op=mybir.AluOpType.add,
                    )
                    e2 = spE.tile([P, H, D], f32, tag="e2")
                    nc.vector.tensor_scalar(
                        out=e2[:], in0=e[:], scalar1=NEG, scalar2=None,
                        op0=mybir.AluOpType.mult,
                    )
                    nc.vector.tensor_tensor(
                        out=e[:], in0=e[:], in1=e2[:], op=mybir.AluOpType.max
                    )
                    negm = spE.tile([P, H], f32, tag="negm")
                    nc.vector.tensor_reduce(
                        out=negm[:], in_=e[:], axis=mybir.AxisListType.X,
                        op=mybir.AluOpType.max, negate=True,
                    )
                    ex = spE.tile([P, H, D], bf16, tag="ex")
                    nc.vector.tensor_tensor(
                        out=ex[:], in0=e[:],
                        in1=negm[:, :, None].to_broadcast([P, H, D]),
                        op=mybir.AluOpType.add,
                    )
                    nc.scalar.activation(out=ex[:], in_=ex[:], func=Act.Exp)
                    den = spE.tile([P, H], f32, tag="den")
                    nc.vector.tensor_reduce(
                        out=den[:], in_=ex[:], axis=mybir.AxisListType.X,
                        op=mybir.AluOpType.add,
                    )
                    rden = spE.tile([P, H], f32, tag="rden")
                    nc.vector.reciprocal(rden[:], den[:])
                    # prod[p, d, h, c] = h_gathered * ex  (bf16)
                    prod = sp.tile([P, D, HC], bf16, tag="prod")
                    g_h = G[:, :, 0:HC].rearrange("p d (h c) -> p d h c", h=H)
                    ex_b = ex.rearrange("p h d -> p d h")[:, :, :, None].to_broadcast(
                        [P, D, H, C]
                    )
                    nc.vector.tensor_tensor(
                        out=prod.rearrange("p d (h c) -> p d h c", h=H),
                        in0=g_h, in1=ex_b, op=mybir.AluOpType.mult,
                    )
                    # tree-reduce over D slots, in place on prod (bf16
                    # until width <= 6, then one f32 level)
                    cur = D
                    while cur > 6:
                        h2 = cur // 2
                        nc.vector.tensor_tensor(
                            out=prod[:, 0:h2, :], in0=prod[:, 0:h2, :],
                            in1=prod[:, h2:2 * h2, :], op=mybir.AluOpType.add,
                        )
                        if cur % 2:
                            nc.vector.tensor_tensor(
                                out=prod[:, 0, :], in0=prod[:, 0, :],
                                in1=prod[:, 2 * h2, :], op=mybir.AluOpType.add,
                            )
                        cur = h2
                    acc = sp.tile([P, HC], f32, tag="accF")
                    if cur == 1:
                        nc.vector.tensor_copy(out=acc[:], in_=prod[:, 0, :])
                    else:
                        nc.vector.tensor_tensor(
                            out=acc[:], in0=prod[:, 0, :], in1=prod[:, 1, :],
                            op=mybir.AluOpType.add,
                        )
                        for j in range(2, cur):
                            nc.vector.tensor_tensor(
                                out=acc[:], in0=acc[:], in1=prod[:, j, :],
                                op=mybir.AluOpType.add,
                            )
                    # xg_raw = acc / den  -> bulk buffer (bf16)
                    nc.vector.tensor_tensor(
                        out=xg_all[:, t, :].rearrange("p (h c) -> p h c", h=H),
                        in0=acc.rearrange("p (h c) -> p h c", h=H),
                        in1=rden[:, :, None].to_broadcast([P, H, C]),
                        op=mybir.AluOpType.mult,
                    )

                # ---- group ELU: xg = elu(xg_raw + bg) in-place (bf16) ----
                ng = g1 - g0
                xs = xg_all[:, g0:g1, :]
                nc.vector.tensor_tensor(
                    out=xs, in0=xs,
                    in1=bg_t[:, None, :].to_broadcast([P, ng, HC]),
                    op=mybir.AluOpType.add,
                )
                tneg = sp.tile([P, GRP * HC], f32, tag="tneg")
                tn = tneg[:, 0:ng * HC].rearrange("p (g c) -> p g c", g=ng)
                nc.vector.tensor_scalar(
                    out=tn, in0=xs, scalar1=0.0, scalar2=None,
                    op0=mybir.AluOpType.min,
                )
                nc.scalar.activation(out=tn, in_=tn, func=Act.Exp)
                nc.vector.tensor_scalar(
                    out=xs, in0=xs, scalar1=0.0, scalar2=None,
                    op0=mybir.AluOpType.max,
                )
                nc.vector.tensor_tensor(
                    out=xs, in0=xs, in1=tn, op=mybir.AluOpType.add,
                )
                nc.vector.tensor_scalar(
                    out=xs, in0=xs, scalar1=1.0, scalar2=None,
                    op0=mybir.AluOpType.subtract,
                )

                # ---- phase C for the group: u = dinv * (xg @ Wc) ----
                for t in range(g0, g1):
                    xwps = ppB.tile([P, HID], f32, tag="mm_psB")
                    for k in range(2):
                        xgT_ps = ppA.tile([P, P], bf16, tag="tr_ps")
                        nc.tensor.transpose(
                            xgT_ps[:], xg_all[:, t, k * P:(k + 1) * P], ident_bf[:]
                        )
                        xgT = sb.tile([P, P], bf16, tag="xgT")
                        nc.scalar.activation(
                            out=xgT[:], in_=xgT_ps[:], func=Act.Copy
                        )
                        nc.tensor.matmul(
                            xwps[:], lhsT=xgT[:], rhs=wc_t[:, k, :],
                            start=(k == 0), stop=(k == 1),
                        )
                    ub = gp.tile([P, HID], bf16, tag="ub")
                    nc.vector.tensor_scalar(
                        out=ub[:], in0=xwps[:], scalar1=dinv_t[:, t:t + 1],
                        scalar2=None, op0=mybir.AluOpType.mult,
                    )
                    nc.sync.dma_start(out=ag_in[t * P:(t + 1) * P, :], in_=ub[:])

            # ---- phase D: AllGather u across cores ----
            nc.gpsimd.collective_compute(
                "AllGather",
                mybir.AluOpType.bypass,
                replica_groups=[list(range(NCORES))],
                ins=[ag_in[:]],
                outs=[ag_out[:]],
            )

            # ---- phase E: GCN via slot-major gathers + classifier ----
            goff = 0
            for t in range(TPC):
                dlo, dhi = D2lo[t], D2hi[t]
                D = dlo + dhi
                w = 8 * D
                idx_t = gu.tile([P, w], i16, tag="gidx2")
                nc.sync.dma_start(out=idx_t[:], in_=gcn_idx[:, goff:goff + w])
                goff += w
                Gu = gu.tile([P, D, HID], bf16, tag="Gu")
                if dlo > 0:
                    nc.gpsimd.dma_gather(
                        out_ap=Gu[:, 0:dlo, :],
                        in_ap=ag_out[:, :],
                        idxs_ap=idx_t[:, 0:8 * dlo],
                        num_idxs=P * dlo,
                        num_idxs_reg=P * dlo,
                        elem_size=HID,
                        single_packet=False,
                        queue_num=qrr[0] % 4,
                    )
                    qrr[0] += 1
                if dhi > 0:
                    nc.gpsimd.dma_gather(
                        out_ap=Gu[:, dlo:D, :],
                        in_ap=ag_out[SPLIT:, :],
                        idxs_ap=idx_t[:, 8 * dlo:w],
                        num_idxs=P * dhi,
                        num_idxs_reg=P * dhi,
                        elem_size=HID,
                        single_packet=False,
                        queue_num=qrr[0] % 4,
                    )
                    qrr[0] += 1
                # tree-reduce over D slots in place (bf16), tail in f32
                cur = D
                while cur > 6:
                    h2 = cur // 2
                    nc.vector.tensor_tensor(
                        out=Gu[:, 0:h2, :], in0=Gu[:, 0:h2, :],
                        in1=Gu[:, h2:2 * h2, :], op=mybir.AluOpType.add,
                    )
                    if cur % 2:
                        nc.vector.tensor_tensor(
                            out=Gu[:, 0, :], in0=Gu[:, 0, :],
                            in1=Gu[:, 2 * h2, :], op=mybir.AluOpType.add,
                        )
                    cur = h2
                uacc = sp.tile([P, HID], f32, tag="uaccF")
                if cur == 1:
                    nc.vector.tensor_copy(out=uacc[:], in_=Gu[:, 0, :])
                else:
                    nc.vector.tensor_tensor(
                        out=uacc[:], in0=Gu[:, 0, :], in1=Gu[:, 1, :],
                        op=mybir.AluOpType.add,
                    )
                    for j in range(2, cur):
                        nc.vector.tensor_tensor(
                            out=uacc[:], in0=uacc[:], in1=Gu[:, j, :],
                            op=mybir.AluOpType.add,
                        )
                # xc = relu(dinv * sum + bc), bf16 for the transpose
                xc = gu.tile([P, HID], bf16, tag="xc")
                nc.vector.tensor_scalar(
                    out=uacc[:], in0=uacc[:], scalar1=dinv2_t[:, t:t + 1],
                    scalar2=None, op0=mybir.AluOpType.mult,
                )
                nc.vector.tensor_tensor(
                    out=uacc[:], in0=uacc[:], in1=bc_t[:], op=mybir.AluOpType.add
                )
                nc.vector.tensor_scalar(
                    out=xc[:], in0=uacc[:], scalar1=0.0, scalar2=None,
                    op0=mybir.AluOpType.max,
                )
                # classifier: transpose xc, then bf16 matmul
                xcT_ps = ppA.tile([P, P], bf16, tag="tr_ps")
                nc.tensor.transpose(xcT_ps[:], xc[:], ident_bf[:])
                xcT = sb.tile([P, P], bf16, tag="xcT")
                nc.scalar.activation(out=xcT[:], in_=xcT_ps[:], func=Act.Copy)
                lps = ppB.tile([P, NCLASS], f32, tag="mm_psB")
                nc.tensor.matmul(lps[:], lhsT=xcT[:], rhs=wlb_t[:],
                                 start=True, stop=True)
                nc.vector.tensor_tensor(
                    out=lg_all[:, t, :], in0=lps[:], in1=bl_t[:],
                    op=mybir.AluOpType.add,
                )

            # ---- bulk log_softmax over all tiles ----
            nmx = sp.tile([P, TPC], f32, tag="nmx")
            nc.vector.tensor_reduce(
                out=nmx[:], in_=lg_all[:], axis=mybir.AxisListType.X,
                op=mybir.AluOpType.max, negate=True,
            )
            nc.vector.tensor_tensor(
                out=lg_all[:], in0=lg_all[:],
                in1=nmx[:, :, None].to_broadcast([P, TPC, NCLASS]),
                op=mybir.AluOpType.add,
            )
            exl = sp.tile([P, TPC, NCLASS], f32, tag="exl")
            nc.scalar.activation(out=exl[:], in_=lg_all[:], func=Act.Exp)
            sume = sp.tile([P, TPC], f32, tag="sume")
            nc.vector.tensor_reduce(
                out=sume[:], in_=exl[:], axis=mybir.AxisListType.X,
                op=mybir.AluOpType.add,
            )
            lns = sp.tile([P, TPC], f32, tag="lns")
            nc.scalar.activation(out=lns[:], in_=sume[:], func=Act.Ln)
            nc.vector.tensor_tensor(
                out=lg_all[:], in0=lg_all[:],
                in1=lns[:, :, None].to_broadcast([P, TPC, NCLASS]),
                op=mybir.AluOpType.subtract,
            )
            nc.sync.dma_start(
                out=out.rearrange("(t p) c -> p t c", p=P), in_=lg_all[:]
            )

    nc.compile()
    return nc


def _prepare(inputs):
    x = np.asarray(inputs["x"], np.float32)
    Wg = np.asarray(inputs["Wg"], np.float32)
    att_src = np.asarray(inputs["att_src"], np.float32)
    att_dst = np.asarray(inputs["att_dst"], np.float32)
    bg = np.asarray(inputs["bg"], np.float32)
    Wc = np.asarray(inputs["Wc"], np.float32)
    bc = np.asarray(inputs["bc"], np.float32)
    Wl = np.asarray(inputs["Wl"], np.float32)
    bl = np.asarray(inputs["bl"], np.float32)
    edge_index = np.asarray(inputs["edge_index"])

    st = _build_structures(edge_index)

    # fold attention vectors into the feature matmul: a_s = x @ (Wg @ As)
    As = np.zeros((HC, H), np.float32)
    Ad = np.zeros((HC, H), np.float32)
    for h in range(H):
        As[h * C:(h + 1) * C, h] = att_src[h]
        Ad[h * C:(h + 1) * C, h] = att_dst[h]
    wg_aug = np.concatenate(
        [Wg, Wg @ As, np.zeros((F_IN, TABW - HC - H), np.float32)], axis=1
    )  # [128, 384], zero-padded so phase A initializes full table rows
    wg_ad = Wg @ Ad                                 # [128, 4]

    x_padT = np.zeros((F_IN, NPAD), np.float32)
    x_padT[:, :N] = x.T

    bf = ml_dtypes.bfloat16
    in_maps = []
    for c in range(NCORES):
        xpT = np.zeros((F_IN, S), np.float32)
        xpT[:, :NPC] = x[st["perm"][c]].T
        dv = np.zeros((P, TPC), np.float32)
        dvp = np.zeros(S, np.float32)
        dvp[:NPC] = st["dinv"][st["perm"][c]]
        dv[:, :] = dvp.reshape(TPC, P).T
        dv2 = np.zeros((P, TPC), np.float32)
        dvp2 = np.zeros(S, np.float32)
        dvp2[:NPC] = st["dinv"][st["perm2"][c]]
        dv2[:, :] = dvp2.reshape(TPC, P).T
        in_maps.append({
            "x_padT": x_padT.astype(bf),
            "x_permT": xpT.astype(bf),
            "dinv_pt": dv,
            "dinv2_pt": dv2,
            "gat_idx": st["gat_idx"][c],
            "gcn_idx": st["gcn_idx"][c],
            "wg_aug": wg_aug.astype(bf),
            "wg_ad": wg_ad.astype(bf),
            "wc": Wc.astype(bf),
            "wl": Wl.astype(bf),
            "bg_b": np.tile(bg[None, :], (P, 1)),
            "bc_b": np.tile(bc[None, :], (P, 1)),
            "bl_b": np.tile(bl[None, :], (P, 1)),
            "ident_bf": np.eye(P, dtype=bf),
            "ident_f": np.eye(P, dtype=np.float32),
        })
    return st, in_maps


def _run(inputs, trace=False, trace_kwargs=None):
    st, in_maps = _prepare(inputs)
    nc = _build_kernel(
        st["Dlo"], st["Dhi"], st["D2lo"], st["D2hi"],
        st["gat_idx"][0].shape[1], st["gcn_idx"][0].shape[1],
    )
    res = run_bass_kernel_spmd(
        nc, in_maps, list(range(NCORES)), trace=trace, **(trace_kwargs or {})
    )
    out = np.empty((N, NCLASS), np.float32)
    for c in range(NCORES):
        out[st["perm2"][c]] = res.results[c]["out"][:NPC]
    return out, res


def kernel(**inputs) -> np.ndarray:
    out, _ = _run(inputs, trace=False)
    return out
